# revision 1
# baseline (speedup 1.0000x reference)
"""GAT (graph attention) message-passing kernel for Trainium2, 8 NeuronCores.

Strategy (graph/data parallel, dst-sharded):
  - Nodes are partitioned across 8 cores by destination id (12500 each).
  - Edges are sharded by dst partition, sorted by (dst-block, src-subtable),
    and padded so every core runs an identical (SPMD) program.
  - Per step, every core projects ALL nodes (h = x @ [W | W@attn_l]) into a
    bf16 row table in its HBM ([h(256) | el(4) | pad] = 768B rows), then
    indirect-gathers h[src] rows per edge (dma_gather), builds one-hot dst
    masks on DVE, computes attention scores (er via a small maskT matmul),
    and accumulates [softmax-denominator | weighted message sum] into
    per-dst-block PSUM with mask matmuls on TensorE.
  - Block epilogue: normalize by the segment sum, head-mean, residual update.
  - Between the 2 conv steps, the updated x (transposed, bf16) is AllGathered
    across the 8 cores.
"""

import os
import math
import numpy as np
import ml_dtypes

import concourse.bass as bass
import concourse.tile as tile
import concourse.mybir as mybir
from concourse import library_config
from concourse.library_overlay import lower_extended_insts
from concourse.bass_utils import run_bass_kernel_spmd

BF16 = mybir.dt.bfloat16
F32 = mybir.dt.float32
I16 = mybir.dt.int16
AF = mybir.ActivationFunctionType
ALU = mybir.AluOpType

NEG_SLOPE = 0.2
STEP = int(os.environ.get("GAT_STEPS", "2"))
SKIP_COLL = bool(int(os.environ.get("GAT_SKIP_COLL", "0")))
SKIP_GATHER = bool(int(os.environ.get("GAT_SKIP_GATHER", "0")))
N_CORES = 8
SB = 4            # blocks per superblock (PSUM accumulators alive at once)
MAX_CALL = 16     # max 128-edge chunks per dma_gather call
GS = 8            # chunks per elementwise batch group
ST_MAX_ROWS = 25000   # subtable rows (int16 gather index limit)

_last_results = None  # BassKernelResults stash for test harness


def _bf(x):
    return np.asarray(x, np.float32).astype(ml_dtypes.bfloat16)


# ----------------------------------------------------------------------------
# host-side preprocessing
# ----------------------------------------------------------------------------

def _plan_and_arrays(src, dst, N):
    """Shard/sort/pad edges; build the shared chunk plan and per-core arrays."""
    Nl = N // N_CORES
    NB = (Nl + 127) // 128
    NSB = (NB + SB - 1) // SB
    NST = max(1, math.ceil(N / ST_MAX_ROWS))
    st_rows = math.ceil(N / NST)

    core = dst // Nl
    percore = []
    for p in range(N_CORES):
        sel = np.nonzero(core == p)[0]
        s = src[sel].astype(np.int64)
        d = (dst[sel] - p * Nl).astype(np.int64)
        blk = d >> 7
        st = s // st_rows
        order = np.lexsort((s, st, blk))
        percore.append((s[order], d[order], blk[order], st[order]))

    counts = np.zeros((N_CORES, NB, NST), np.int64)
    for p in range(N_CORES):
        _, _, blk, st = percore[p]
        np.add.at(counts, (p, blk, st), 1)
    nchunks = (counts.max(axis=0) + 127) // 128          # [NB, NST]

    # canonical chunk emission order
    chunk_meta = []   # (isb, st, b) per chunk
    calls = []        # (st, chunk_lo, n_chunks)
    for isb in range(NSB):
        blocks = range(isb * SB, min((isb + 1) * SB, NB))
        for st in range(NST):
            run_lo = len(chunk_meta)
            for b in blocks:
                for _ in range(int(nchunks[b, st])):
                    chunk_meta.append((isb, st, b))
            n = len(chunk_meta) - run_lo
            o = run_lo
            while n > 0:
                take = min(n, MAX_CALL)
                calls.append((st, o, take))
                o += take
                n -= take
    NCH = len(chunk_meta)

    # first/last chunk index per (isb, b) for PSUM start/stop flags
    first = {}
    last = {}
    for ci, (isb, st, b) in enumerate(chunk_meta):
        key = (isb, b)
        if key not in first:
            first[key] = ci
        last[key] = ci

    # per-core edge arrays in padded chunk order
    idx_all = np.zeros((N_CORES, NCH * 128), np.int16)
    doff_all = np.full((N_CORES, NCH * 128), 255.0, np.float32)
    for p in range(N_CORES):
        s, d, blk, st = percore[p]
        # build run boundaries of the (blk, st)-sorted edge list
        runs = {}
        i = 0
        M = len(s)
        while i < M:
            k = (blk[i], st[i])
            j = i
            while j < M and blk[j] == k[0] and st[j] == k[1]:
                j += 1
            runs[k] = (i, j)
            i = j
        cursor = {k: v[0] for k, v in runs.items()}
        for ci, (isb, t, b) in enumerate(chunk_meta):
            base = ci * 128
            k = (b, t)
            if k in runs:
                lo = cursor[k]
                hi = min(lo + 128, runs[k][1])
                n = hi - lo
                cursor[k] = hi
                if n > 0:
                    idx_all[p, base:base + n] = (s[lo:hi] - t * st_rows).astype(np.int16)
                    doff_all[p, base:base + n] = (d[lo:hi] - b * 128).astype(np.float32)
        for k, (lo, hi) in runs.items():
            assert cursor[k] == hi, "edge run not fully consumed"

    # gather-call wrapped idx layout: per call [16, n/16], concat on free axis
    idxw_cols = NCH * 8
    idx_wrapped = np.zeros((N_CORES, 16, idxw_cols), np.int16)
    col = 0
    call_cols = []
    for (t, lo, nch) in calls:
        n = nch * 128
        for p in range(N_CORES):
            seg = idx_all[p, lo * 128: lo * 128 + n]
            idx_wrapped[p, :, col:col + n // 16] = seg.reshape(-1, 16).T
        call_cols.append(col)
        col += n // 16
    assert col == idxw_cols

    # dstoff [128, NCH]: partition = edge-in-chunk
    doff = doff_all.reshape(N_CORES, NCH, 128).transpose(0, 2, 1)
    # dstrep [128, 4*NCH]: dstrep[p, 4c+j] = doff_edge(c, 32j + p%32)
    j_idx = np.arange(4)
    p_idx = np.arange(128)
    e_idx = (32 * j_idx[None, :] + (p_idx % 32)[:, None])      # [128, 4]
    dstrep = np.empty((N_CORES, 128, 4 * NCH), np.float32)
    for p in range(N_CORES):
        d3 = doff_all[p].reshape(NCH, 128)                      # [NCH, 128e]
        rep = d3[:, e_idx]                                      # [NCH, 128, 4]
        dstrep[p] = rep.transpose(1, 0, 2).reshape(128, NCH * 4)

    groups = []
    for (t, lo, nch) in calls:
        g = lo
        while g < lo + nch:
            take = min(GS, lo + nch - g)
            groups.append((t, lo, g, take))  # (st, call_lo, group_lo, size)
            g += take

    return dict(Nl=Nl, NB=NB, NSB=NSB, NST=NST, st_rows=st_rows, NCH=NCH,
                chunk_meta=chunk_meta, calls=calls, call_cols=call_cols,
                groups=groups, first=first, last=last,
                idx_wrapped=idx_wrapped, dstoff=doff, dstrep=dstrep,
                idxw_cols=idxw_cols)


# ----------------------------------------------------------------------------
# device program
# ----------------------------------------------------------------------------

def _split_multi_waits(nc):
    """walrus codegen only accepts one sync-wait per instruction; hoist any
    extra waits onto same-engine NOPs inserted right before the instruction."""
    n_id = 0
    for f in nc.m.functions:
        for blk in f.blocks:
            out = []
            for ins in blk.instructions:
                si = ins.sync_info
                if si is not None and len(si.on_wait) > 1 \
                        and ins.engine is not None:
                    waits = list(si.on_wait)
                    for w in waits[:-1]:
                        nop = mybir.InstNoOp(name=f"I-wsplit-{n_id}", ins=[],
                                             outs=[])
                        n_id += 1
                        nop.engine = ins.engine
                        nop.sync_info = mybir.SyncInfo(on_wait=[w],
                                                       on_update=[])
                        nc.inst_map[nop.name] = nop
                        out.append(nop)
                    ins.sync_info = mybir.SyncInfo(on_wait=[waits[-1]],
                                                   on_update=list(si.on_update))
                out.append(ins)
            blk.instructions = out

def _ap(base, *dims):
    """Rebuild AP with the same tensor/offset/partition dim, custom free dims."""
    return bass.AP(base.tensor, base.offset,
                   [list(base.ap[0])] + [list(d) for d in dims])


def _build(meta, N, D, H):
    Nl, NB, NSB, NST = meta["Nl"], meta["NB"], meta["NSB"], meta["NST"]
    st_rows = meta["st_rows"]
    NBP = NB * 128
    HD = H * D            # 256
    RW = HD + H           # 260 (h | el)
    TW = ((RW * 2 + 255) // 256) * 128  # 384 elems bf16 -> 768B rows

    nc = bass.Bass("TRN2", target_bir_lowering=False, debug=False,
                   enable_asserts=False, num_devices=N_CORES)

    # ---- DRAM tensors
    xT_in = nc.dram_tensor("xT_in", [D, N], BF16, kind="ExternalInput")
    xTl_in = nc.dram_tensor("xTl_in", [D, NBP], BF16, kind="ExternalInput")
    x_in = nc.dram_tensor("x_in", [128, NB, D], F32, kind="ExternalInput")
    c0_in = nc.dram_tensor("c0_in", [128, NB, D], F32, kind="ExternalInput")
    waug_in = nc.dram_tensor("waug_in", [D, RW], BF16, kind="ExternalInput")
    wr_in = nc.dram_tensor("wr_in", [D, H], BF16, kind="ExternalInput")
    iota_in = nc.dram_tensor("iota_in", [128, 128], BF16, kind="ExternalInput")
    itld_in = nc.dram_tensor("itld_in", [128, 32], BF16, kind="ExternalInput")
    ident_in = nc.dram_tensor("ident_in", [128, 128], BF16, kind="ExternalInput")
    scal_in = nc.dram_tensor("scal_in", [128, 4], F32, kind="ExternalInput")
    idx_in = nc.dram_tensor("idx_in", [128, meta["idxw_cols"]], I16,
                            kind="ExternalInput")
    doff_in = nc.dram_tensor("doff_in", [128, meta["NCH"]], BF16,
                             kind="ExternalInput")
    drep_in = nc.dram_tensor("drep_in", [128, 4 * meta["NCH"]], BF16,
                             kind="ExternalInput")

    table = nc.dram_tensor("table", [N, TW], BF16, kind="Internal")
    x_mid = nc.dram_tensor("x_mid", [128, NB, D], F32, kind="Internal")
    xT_sh = nc.dram_tensor("xT_sh", [D, NBP], BF16, kind="Internal")
    xT_ag = nc.dram_tensor("xT_ag", [D * N_CORES, NBP], BF16, kind="Internal",
                           addr_space="Shared")
    x_out = nc.dram_tensor("x_out", [Nl, D], F32, kind="ExternalOutput")

    from contextlib import ExitStack
    with tile.TileContext(nc) as tc, ExitStack() as es_:
        nc.gpsimd.load_library(library_config.mlp)
        cp = es_.enter_context(tc.tile_pool(name="consts", bufs=1))
        pools = {}
        for nm, bufs in [("xt", 4), ("rows", 3), ("mask", 2), ("rhs", 3),
                         ("sm", 3), ("tbl", 4), ("blk", 3), ("big", 2)]:
            pools[nm] = es_.enter_context(tc.tile_pool(name=nm, bufs=bufs))
        pA = es_.enter_context(tc.tile_pool(name="pacc", bufs=1, space="PSUM"))
        pB = es_.enter_context(tc.tile_pool(name="per8", bufs=2, space="PSUM"))
        pC = es_.enter_context(tc.tile_pool(name="ppj", bufs=2, space="PSUM"))

        # ---- load constants
        iota_t = cp.tile([128, 128], BF16, tag="iota")
        itld_t = cp.tile([128, 32], BF16, tag="itld")
        ident_t = cp.tile([128, 128], BF16, tag="ident")
        waug_t = cp.tile([D, RW], BF16, tag="waug")
        wr_t = cp.tile([D, H], BF16, tag="wr")
        scal_t = cp.tile([128, 4], F32, tag="scal")
        idx_t = cp.tile([128, meta["idxw_cols"]], I16, tag="idx")
        doff_t = cp.tile([128, meta["NCH"]], BF16, tag="doff")
        drep_t = cp.tile([128, 4 * meta["NCH"]], BF16, tag="drep")
        for t, s in [(iota_t, iota_in), (itld_t, itld_in), (ident_t, ident_in),
                     (waug_t, waug_in), (wr_t, wr_in), (scal_t, scal_in),
                     (idx_t, idx_in), (doff_t, doff_in), (drep_t, drep_in)]:
            nc.sync.dma_start(t[:], s.ap()[:])

        # zero-fill the table's pad columns once (the gather reads full
        # 768B rows; compute never touches the pad, but it must be finite)
        PAD = TW - RW
        zt = cp.tile([128, PAD], BF16, tag="zpad")
        nc.vector.memset(zt[:], 0)
        nrep = N // 128
        tap = table.ap()
        nc.sync.dma_start(
            bass.AP(tap.tensor, RW, [[TW, 128], [TW * 128, nrep], [1, PAD]]),
            _ap(zt[:], [0, nrep], [1, PAD]))

        tails = {NB - 1: Nl - 128 * (NB - 1)}
        nidx_regs = {}

        def nidx_reg(n):
            if n not in nidx_regs:
                nidx_regs[n] = nc.gpsimd.to_reg(n)
            return nidx_regs[n]

        for step in range(STEP):
            # ------------------------------------------------ projection
            eng_flip = 0
            for r in range(N_CORES):
                for t in range(NB):
                    o = 128 * t
                    w = min(128, Nl - o)
                    g0 = r * Nl + o
                    xt = pools["xt"].tile([D, 128], BF16, tag="projlhs")
                    if step == 0:
                        nc.sync.dma_start(xt[:, :w], xT_in.ap()[:, g0:g0 + w])
                    else:
                        nc.sync.dma_start(
                            xt[:, :w], xT_ag.ap()[D * r:D * (r + 1), o:o + w])
                    pp = pC.tile([128, RW], F32, tag="pj")
                    nc.tensor.matmul(pp[:w, :], xt[:, :w], waug_t[:],
                                     start=True, stop=True)
                    tb = pools["tbl"].tile([128, RW], BF16, tag="tbl")
                    if eng_flip % 2 == 0:
                        nc.vector.tensor_copy(tb[:w, :], pp[:w, :])
                    else:
                        nc.scalar.activation(tb[:w, :], pp[:w, :], AF.Copy)
                    eng_flip += 1
                    nc.sync.dma_start(table.ap()[g0:g0 + w, 0:RW], tb[:w, :])

            # ------------------------------------------------ gather + attn
            x_src = x_in if step == 0 else x_mid
            xt_src = xTl_in if step == 0 else xT_sh
            call_i = 0
            group_i = 0
            for isb in range(NSB):
                blocks = list(range(isb * SB, min((isb + 1) * SB, NB)))
                nb = len(blocks)
                b0 = blocks[0]
                acc = pA.tile([128, SB, 512], F32, tag="acc")
                x4 = pools["blk"].tile([128, SB, D], F32, tag="x4")
                c04 = pools["blk"].tile([128, SB, D], F32, tag="c04")
                nc.sync.dma_start(x4[:, :nb, :], x_src.ap()[:, b0:b0 + nb, :])
                nc.sync.dma_start(c04[:, :nb, :], c0_in.ap()[:, b0:b0 + nb, :])
                # x4p = (1-alpha) * x4 + c0
                x4p = pools["blk"].tile([128, SB, D], F32, tag="x4p")
                nc.vector.scalar_tensor_tensor(
                    x4p[:, :nb, :], x4[:, :nb, :], scal_t[:, 0:1],
                    c04[:, :nb, :], op0=ALU.mult, op1=ALU.add)
                er_sb = {}
                for j, b in enumerate(blocks):
                    xtb = pools["xt"].tile([D, 128], BF16, tag="erlhs")
                    nc.sync.dma_start(xtb[:], xt_src.ap()[:, 128 * b:128 * (b + 1)])
                    nc.tensor.matmul(acc[:, j, 264:264 + H], xtb[:], wr_t[:],
                                     start=True, stop=True)
                    es = pools["sm"].tile([128, H], BF16, tag="erblk%d" % j)
                    nc.scalar.activation(es[:], acc[:, j, 264:264 + H], AF.Copy)
                    er_sb[b] = es

                # walk this superblock's calls/groups/chunks
                while call_i < len(meta["calls"]):
                    st, lo, nch = meta["calls"][call_i]
                    if lo >= len(meta["chunk_meta"]) or \
                       meta["chunk_meta"][lo][0] != isb:
                        break
                    n = nch * 128
                    rows = pools["rows"].tile([128, MAX_CALL, TW], BF16,
                                              tag="rows")
                    icol = meta["call_cols"][call_i]
                    rows_ap = _ap(rows[:], [TW, nch], [1, TW])
                    tbl_ap = table.ap()[st * st_rows:
                                        min((st + 1) * st_rows, N), :]
                    if not SKIP_GATHER:
                        nc.gpsimd.dma_gather(
                            rows_ap, tbl_ap, idx_t[:, icol:icol + n // 16],
                            num_idxs=n, num_idxs_reg=nidx_reg(n), elem_size=TW,
                            single_packet=False)
                    call_i += 1

                    while group_i < len(meta["groups"]):
                        gst, glo_call, g, gs = meta["groups"][group_i]
                        if glo_call != lo:
                            break
                        group_i += 1
                        cc0 = g - lo   # chunk offset within call
                        # mask [128, gs, 128]
                        m8 = pools["mask"].tile([128, GS, 128], BF16, tag="m8")
                        nc.vector.tensor_tensor(
                            _ap(m8[:], [128, gs], [1, 128]),
                            _ap(iota_t[:], [0, gs], [1, 128]),
                            _ap(doff_t[:, g:g + gs], [1, gs], [0, 128]),
                            op=ALU.is_equal)
                        mt8 = pools["mask"].tile([128, GS, 128], BF16, tag="mt8")
                        nc.vector.tensor_tensor(
                            _ap(mt8[:], [128, gs], [1, 128]),
                            _ap(drep_t[:, 4 * g:4 * (g + gs)],
                                [4, gs], [1, 4], [0, 32]),
                            _ap(itld_t[:], [0, gs], [0, 4], [1, 32]),
                            op=ALU.is_equal)
                        mT8 = pools["mask"].tile([128, GS, 128], BF16, tag="mT8")
                        nc.vector.transpose(
                            _ap(mT8[:], [1, gs * 128]),
                            _ap(mt8[:], [1, gs * 128]))
                        er8 = pB.tile([128, GS * H], F32, tag="er8")
                        for k in range(gs):
                            ci = g + k
                            _, _, b = meta["chunk_meta"][ci]
                            nc.tensor.matmul(er8[:, H * k:H * (k + 1)],
                                             mT8[:, k, :], er_sb[b],
                                             start=True, stop=False)
                            nc.tensor.matmul(er8[:, H * k:H * (k + 1)],
                                             ident_t[:],
                                             rows[:, cc0 + k, HD:HD + H],
                                             start=False, stop=True)
                        t8 = pools["sm"].tile([128, GS * H], BF16, tag="t8")
                        nc.scalar.activation(t8[:, :gs * H], er8[:, :gs * H],
                                             AF.Copy)
                        lr8 = pools["sm"].tile([128, GS * H], BF16, tag="lr8")
                        nc.vector.scalar_tensor_tensor(
                            lr8[:, :gs * H], t8[:, :gs * H], NEG_SLOPE,
                            t8[:, :gs * H], op0=ALU.mult, op1=ALU.max)
                        rhs8 = pools["rhs"].tile([128, GS, RW], BF16, tag="rhs8")
                        nc.scalar.activation(
                            _ap(rhs8[:], [RW, gs], [1, H]),
                            _ap(lr8[:], [H, gs], [1, H]), AF.Exp)
                        nc.vector.tensor_tensor(
                            _ap(rhs8[:, :, H:RW], [RW, gs], [D, H], [1, D]),
                            _ap(rows[:, cc0:cc0 + gs, 0:HD],
                                [TW, gs], [D, H], [1, D]),
                            _ap(rhs8[:], [RW, gs], [1, H], [0, D]),
                            op=ALU.mult)
                        for k in range(gs):
                            ci = g + k
                            _, _, b = meta["chunk_meta"][ci]
                            j = b - b0
                            nc.tensor.matmul(
                                acc[:, j, 0:RW], m8[:, k, :], rhs8[:, k, :],
                                start=(meta["first"][(isb, b)] == ci),
                                stop=(meta["last"][(isb, b)] == ci),
                                skip_group_check=True)

                # ---- superblock epilogue (batched over blocks)
                smax = pools["sm"].tile([128, SB * H], F32, tag="smax")
                nc.vector.tensor_scalar(
                    _ap(smax[:], [H, nb], [1, H]),
                    _ap(acc[:], [512, nb], [1, H]),
                    1e-30, None, op0=ALU.max)
                srec = pools["sm"].tile([128, SB * H], F32, tag="srec")
                nc.vector.reciprocal(srec[:, :nb * H], smax[:, :nb * H])
                srec2 = pools["sm"].tile([128, SB * H], F32, tag="srec2")
                nc.vector.tensor_scalar(
                    srec2[:, :nb * H], srec[:, :nb * H], scal_t[:, 1:2], None,
                    op0=ALU.mult)
                onorm = pools["big"].tile([128, SB, H, D], F32, tag="onorm")
                nc.vector.tensor_tensor(
                    _ap(onorm[:], [H * D, nb], [D, H], [1, D]),
                    _ap(acc[:, :, H:RW], [512, nb], [D, H], [1, D]),
                    _ap(srec2[:], [H, nb], [1, H], [0, D]),
                    op=ALU.mult)
                red = pools["blk"].tile([128, SB, D], F32, tag="red")
                nc.vector.tensor_reduce(
                    _ap(red[:], [D, nb], [1, D]),
                    _ap(onorm[:], [H * D, nb], [1, D], [D, H]),
                    axis=mybir.AxisListType.X, op=ALU.add)
                xn = pools["blk"].tile([128, SB, D], F32, tag="xn")
                nc.vector.tensor_add(xn[:, :nb, :], x4p[:, :nb, :],
                                     red[:, :nb, :])
                if step < STEP - 1:
                    xnb = pools["blk"].tile([128, SB, D], BF16, tag="xnb")
                    nc.vector.tensor_copy(xnb[:, :nb, :], xn[:, :nb, :])
                    nc.sync.dma_start(x_mid.ap()[:, b0:b0 + nb, :],
                                      xn[:, :nb, :])
                    for j, b in enumerate(blocks):
                        tp = pC.tile([D, 128], BF16, tag="pj")
                        nc.tensor.transpose(tp[:], xnb[:, j, :], ident_t[:])
                        xts = pools["sm"].tile([D, 128], BF16, tag="xts")
                        nc.scalar.activation(xts[:], tp[:], AF.Copy)
                        nc.sync.dma_start(
                            xT_sh.ap()[:, 128 * b:128 * (b + 1)], xts[:])
                else:
                    for j, b in enumerate(blocks):
                        w = tails.get(b, 128)
                        nc.sync.dma_start(x_out.ap()[128 * b:128 * b + w, :],
                                          xn[:w, j, :])
            assert call_i == len(meta["calls"]) and \
                group_i == len(meta["groups"])

            if step == 0 and STEP > 1 and not SKIP_COLL:
                nc.gpsimd.collective_compute(
                    "AllGather", ALU.bypass,
                    replica_groups=[list(range(N_CORES))],
                    ins=[xT_sh.ap()[:]], outs=[xT_ag.ap()[:]])

    _split_multi_waits(nc)
    lower_extended_insts(nc)
    return nc


# ----------------------------------------------------------------------------
# entry point
# ----------------------------------------------------------------------------

def kernel(x, x0, src, dst, W, attn_l, attn_r, alpha, lamda, **kw):
    global _last_results
    x = np.asarray(x, np.float32)
    x0 = np.asarray(x0, np.float32)
    src = np.asarray(src)
    dst = np.asarray(dst)
    W = np.asarray(W, np.float32)
    attn_l = np.asarray(attn_l, np.float32)
    attn_r = np.asarray(attn_r, np.float32)
    alpha_f = float(np.asarray(alpha))
    lamda_f = float(np.asarray(lamda))

    N, D = x.shape
    H = attn_l.shape[0]
    assert N % N_CORES == 0
    meta = _plan_and_arrays(src, dst, N)
    Nl, NB = meta["Nl"], meta["NB"]
    NBP = NB * 128

    nc = _build(meta, N, D, H)

    # host-side weight prep
    W3 = W.reshape(D, H, D)
    WL = np.einsum("khd,hd->kh", W3, attn_l)
    WR = np.einsum("khd,hd->kh", W3, attn_r)
    waug = _bf(np.concatenate([W, WL], axis=1))
    wr = _bf(WR)
    iota = _bf(np.tile(np.arange(128, dtype=np.float32)[None, :], (128, 1)))
    itld = _bf((32 * (np.arange(128)[:, None] // 32)
                + np.arange(32)[None, :]).astype(np.float32))
    ident = _bf(np.eye(128, dtype=np.float32))
    scal = np.zeros((128, 4), np.float32)
    scal[:, 0] = 1.0 - alpha_f
    scal[:, 1] = alpha_f / H
    c0 = (alpha_f * lamda_f) * x0

    xT = _bf(x.T).copy()                      # [D, N]
    in_maps = []
    for p in range(N_CORES):
        lo = p * Nl
        xl = np.zeros((NBP, D), np.float32)
        xl[:Nl] = x[lo:lo + Nl]
        c0l = np.zeros((NBP, D), np.float32)
        c0l[:Nl] = c0[lo:lo + Nl]
        in_maps.append({
            "xT_in": np.ascontiguousarray(xT),
            "xTl_in": np.ascontiguousarray(_bf(xl.T)),
            "x_in": np.ascontiguousarray(
                xl.reshape(NB, 128, D).transpose(1, 0, 2)),
            "c0_in": np.ascontiguousarray(
                c0l.reshape(NB, 128, D).transpose(1, 0, 2)),
            "waug_in": waug, "wr_in": wr,
            "iota_in": iota, "itld_in": itld, "ident_in": ident,
            "scal_in": scal,
            "idx_in": np.ascontiguousarray(
                np.tile(meta["idx_wrapped"][p], (8, 1))),
            "doff_in": np.ascontiguousarray(_bf(meta["dstoff"][p])),
            "drep_in": np.ascontiguousarray(_bf(meta["dstrep"][p])),
        })

    trace = bool(int(os.environ.get("GAT_TRACE", "0")))
    res = run_bass_kernel_spmd(nc, in_maps, core_ids=list(range(N_CORES)),
                               trace=trace,
                               trace_cores=list(range(N_CORES)) if trace else None,
                               stitch_traces=False)
    _last_results = res
    out = np.concatenate([res.results[p]["x_out"] for p in range(N_CORES)],
                         axis=0)
    return out.astype(np.float32)



# revision 3
# speedup vs baseline: 1.1377x; 1.1377x over previous
"""GAT (graph attention) message-passing kernel for Trainium2, 8 NeuronCores.

Strategy (graph/data parallel, dst-sharded):
  - Nodes are partitioned across 8 cores by destination id (12500 each).
  - Edges are sharded by dst partition, sorted by (dst-block, src-subtable),
    and padded so every core runs an identical (SPMD) program.
  - Per step, every core projects ALL nodes (h = x @ [W | W@attn_l]) into an
    fp8 row table in its HBM ([h(256B) | el(4B) | pad] @ 512B stride), then
    indirect-gathers h[src] rows per edge (dma_gather, 4 SWDGE queues round
    robin so descriptor generation runs on all 4 Q7 core pairs), computes
    attention scores (er via host-precomputed transposed one-hot masks
    streamed from HBM as fp8 + gathered el added on DVE), and accumulates
    [softmax-denominator | weighted message sum] into per-dst-block PSUM
    with mask matmuls on TensorE.  The dst one-hot masks are built on DVE
    with per-chunk tensor_scalar(is_equal) (4x mode).
  - Block epilogue: normalize by the segment sum, head-mean, residual update.
  - Between the 2 conv steps, the updated x (transposed, bf16) is AllGathered
    across the 8 cores.
"""

import os
import math
import numpy as np
import ml_dtypes

import concourse.bass as bass
import concourse.tile as tile
import concourse.mybir as mybir
from concourse import library_config
from concourse.library_overlay import lower_extended_insts
from concourse.bass_utils import run_bass_kernel_spmd

BF16 = mybir.dt.bfloat16
F32 = mybir.dt.float32
F8 = mybir.dt.float8e4
I16 = mybir.dt.int16
AF = mybir.ActivationFunctionType
ALU = mybir.AluOpType

NEG_SLOPE = 0.2
STEP = int(os.environ.get("GAT_STEPS", "2"))
SKIP_COLL = bool(int(os.environ.get("GAT_SKIP_COLL", "0")))
SKIP_GATHER = bool(int(os.environ.get("GAT_SKIP_GATHER", "0")))
N_QUEUES = int(os.environ.get("GAT_QUEUES", "4"))
SINGLE_PACKET = bool(int(os.environ.get("GAT_SINGLE_PACKET", "0")))
N_CORES = 8
SB = 4            # blocks per superblock (PSUM accumulators alive at once)
MAX_CALL = 16     # max 128-edge chunks per dma_gather call
GS = 8            # chunks per elementwise batch group
ST_MAX_ROWS = 25000   # subtable rows (int16 gather index limit)

_last_results = None  # BassKernelResults stash for test harness


def _bf(x):
    return np.asarray(x, np.float32).astype(ml_dtypes.bfloat16)


def _f8(x):
    return np.asarray(x, np.float32).astype(ml_dtypes.float8_e4m3fn)


# ----------------------------------------------------------------------------
# host-side preprocessing
# ----------------------------------------------------------------------------

def _plan_and_arrays(src, dst, N):
    """Shard/sort/pad edges; build the shared chunk plan and per-core arrays."""
    Nl = N // N_CORES
    NB = (Nl + 127) // 128
    NSB = (NB + SB - 1) // SB
    NST = max(1, math.ceil(N / ST_MAX_ROWS))
    st_rows = math.ceil(N / NST)

    core = dst // Nl
    percore = []
    for p in range(N_CORES):
        sel = np.nonzero(core == p)[0]
        s = src[sel].astype(np.int64)
        d = (dst[sel] - p * Nl).astype(np.int64)
        blk = d >> 7
        st = s // st_rows
        order = np.lexsort((s, st, blk))
        percore.append((s[order], d[order], blk[order], st[order]))

    counts = np.zeros((N_CORES, NB, NST), np.int64)
    for p in range(N_CORES):
        _, _, blk, st = percore[p]
        np.add.at(counts, (p, blk, st), 1)
    nchunks = (counts.max(axis=0) + 127) // 128          # [NB, NST]

    # canonical chunk emission order
    chunk_meta = []   # (isb, st, b) per chunk
    calls = []        # (st, chunk_lo, n_chunks)
    for isb in range(NSB):
        blocks = range(isb * SB, min((isb + 1) * SB, NB))
        for st in range(NST):
            run_lo = len(chunk_meta)
            for b in blocks:
                for _ in range(int(nchunks[b, st])):
                    chunk_meta.append((isb, st, b))
            n = len(chunk_meta) - run_lo
            o = run_lo
            while n > 0:
                take = min(n, MAX_CALL)
                calls.append((st, o, take))
                o += take
                n -= take
    NCH = len(chunk_meta)

    # first/last chunk index per (isb, b) for PSUM start/stop flags
    first = {}
    last = {}
    for ci, (isb, st, b) in enumerate(chunk_meta):
        key = (isb, b)
        if key not in first:
            first[key] = ci
        last[key] = ci

    # per-core edge arrays in padded chunk order
    idx_all = np.zeros((N_CORES, NCH * 128), np.int16)
    doff_all = np.full((N_CORES, NCH * 128), 255.0, np.float32)
    for p in range(N_CORES):
        s, d, blk, st = percore[p]
        # build run boundaries of the (blk, st)-sorted edge list
        runs = {}
        i = 0
        M = len(s)
        while i < M:
            k = (blk[i], st[i])
            j = i
            while j < M and blk[j] == k[0] and st[j] == k[1]:
                j += 1
            runs[k] = (i, j)
            i = j
        cursor = {k: v[0] for k, v in runs.items()}
        for ci, (isb, t, b) in enumerate(chunk_meta):
            base = ci * 128
            k = (b, t)
            if k in runs:
                lo = cursor[k]
                hi = min(lo + 128, runs[k][1])
                n = hi - lo
                cursor[k] = hi
                if n > 0:
                    idx_all[p, base:base + n] = (s[lo:hi] - t * st_rows).astype(np.int16)
                    doff_all[p, base:base + n] = (d[lo:hi] - b * 128).astype(np.float32)
        for k, (lo, hi) in runs.items():
            assert cursor[k] == hi, "edge run not fully consumed"

    # gather-call wrapped idx layout: per call [16, n/16], concat on free axis
    idxw_cols = NCH * 8
    idx_wrapped = np.zeros((N_CORES, 16, idxw_cols), np.int16)
    col = 0
    call_cols = []
    for (t, lo, nch) in calls:
        n = nch * 128
        for p in range(N_CORES):
            seg = idx_all[p, lo * 128: lo * 128 + n]
            idx_wrapped[p, :, col:col + n // 16] = seg.reshape(-1, 16).T
        call_cols.append(col)
        col += n // 16
    assert col == idxw_cols

    # dstoff [128, NCH]: partition = edge-in-chunk
    doff = doff_all.reshape(N_CORES, NCH, 128).transpose(0, 2, 1)

    groups = []
    for (t, lo, nch) in calls:
        g = lo
        while g < lo + nch:
            take = min(GS, lo + nch - g)
            groups.append((t, lo, g, take))  # (st, call_lo, group_lo, size)
            g += take

    return dict(Nl=Nl, NB=NB, NSB=NSB, NST=NST, st_rows=st_rows, NCH=NCH,
                chunk_meta=chunk_meta, calls=calls, call_cols=call_cols,
                groups=groups, first=first, last=last,
                idx_wrapped=idx_wrapped, dstoff=doff, doff_raw=doff_all,
                idxw_cols=idxw_cols)


# ----------------------------------------------------------------------------
# device program
# ----------------------------------------------------------------------------

def _split_multi_waits(nc):
    """walrus codegen only accepts one sync-wait per instruction; hoist any
    extra waits onto same-engine NOPs inserted right before the instruction."""
    n_id = 0
    for f in nc.m.functions:
        for blk in f.blocks:
            out = []
            for ins in blk.instructions:
                si = ins.sync_info
                if si is not None and len(si.on_wait) > 1 \
                        and ins.engine is not None:
                    waits = list(si.on_wait)
                    for w in waits[:-1]:
                        nop = mybir.InstNoOp(name=f"I-wsplit-{n_id}", ins=[],
                                             outs=[])
                        n_id += 1
                        nop.engine = ins.engine
                        nop.sync_info = mybir.SyncInfo(on_wait=[w],
                                                       on_update=[])
                        nc.inst_map[nop.name] = nop
                        out.append(nop)
                    ins.sync_info = mybir.SyncInfo(on_wait=[waits[-1]],
                                                   on_update=list(si.on_update))
                out.append(ins)
            blk.instructions = out

def _ap(base, *dims):
    """Rebuild AP with the same tensor/offset/partition dim, custom free dims."""
    return bass.AP(base.tensor, base.offset,
                   [list(base.ap[0])] + [list(d) for d in dims])


def _build(meta, N, D, H):
    Nl, NB, NSB, NST = meta["Nl"], meta["NB"], meta["NSB"], meta["NST"]
    st_rows = meta["st_rows"]
    NBP = NB * 128
    HD = H * D            # 256
    RW = HD + H           # 260 elems (h | el), fp8 -> 260B used
    TW = 512              # fp8 table row stride: 512B (gather elem size)

    nc = bass.Bass("TRN2", target_bir_lowering=False, debug=False,
                   enable_asserts=False, num_devices=N_CORES,
                   num_swdge_queues=N_QUEUES)

    # ---- DRAM tensors
    xT_in = nc.dram_tensor("xT_in", [D, N], BF16, kind="ExternalInput")
    xTl_in = nc.dram_tensor("xTl_in", [D, NBP], BF16, kind="ExternalInput")
    x_in = nc.dram_tensor("x_in", [128, NB, D], F32, kind="ExternalInput")
    c0_in = nc.dram_tensor("c0_in", [128, NB, D], F32, kind="ExternalInput")
    waug_in = nc.dram_tensor("waug_in", [D, RW], BF16, kind="ExternalInput")
    wr_in = nc.dram_tensor("wr_in", [D, H], BF16, kind="ExternalInput")
    iota_in = nc.dram_tensor("iota_in", [128, 128], BF16, kind="ExternalInput")
    ident_in = nc.dram_tensor("ident_in", [128, 128], BF16, kind="ExternalInput")
    scal_in = nc.dram_tensor("scal_in", [128, 4], F32, kind="ExternalInput")
    idx_in = nc.dram_tensor("idx_in", [128, meta["idxw_cols"]], I16,
                            kind="ExternalInput")
    doff_in = nc.dram_tensor("doff_in", [128, meta["NCH"]], F32,
                             kind="ExternalInput")
    mt8_in = nc.dram_tensor("mt8_in", [128, meta["NCH"] * 128], F8,
                            kind="ExternalInput")

    table = nc.dram_tensor("table", [N, TW], F8, kind="Internal")
    x_mid = nc.dram_tensor("x_mid", [128, NB, D], F32, kind="Internal")
    xT_sh = nc.dram_tensor("xT_sh", [D, NBP], BF16, kind="Internal")
    xT_ag = nc.dram_tensor("xT_ag", [D * N_CORES, NBP], BF16, kind="Internal",
                           addr_space="Shared")
    x_out = nc.dram_tensor("x_out", [Nl, D], F32, kind="ExternalOutput")

    from contextlib import ExitStack
    with tile.TileContext(nc) as tc, ExitStack() as es_:
        nc.gpsimd.load_library(library_config.mlp)
        cp = es_.enter_context(tc.tile_pool(name="consts", bufs=1))
        pools = {}
        for nm, bufs in [("xt", 4), ("rows", 6), ("mask", 3), ("m8p", 2),
                         ("rhs", 3), ("sm", 4), ("tbl", 4), ("blk", 3),
                         ("big", 2)]:
            pools[nm] = es_.enter_context(tc.tile_pool(name=nm, bufs=bufs))
        pA = es_.enter_context(tc.tile_pool(name="pacc", bufs=1, space="PSUM"))
        pB = es_.enter_context(tc.tile_pool(name="per8", bufs=2, space="PSUM"))
        pC = es_.enter_context(tc.tile_pool(name="ppj", bufs=2, space="PSUM"))

        # ---- load constants
        iota_t = cp.tile([128, 128], BF16, tag="iota")
        ident_t = cp.tile([128, 128], BF16, tag="ident")
        waug_t = cp.tile([D, RW], BF16, tag="waug")
        wr_t = cp.tile([D, H], BF16, tag="wr")
        scal_t = cp.tile([128, 4], F32, tag="scal")
        idx_t = cp.tile([128, meta["idxw_cols"]], I16, tag="idx")
        doff_t = cp.tile([128, meta["NCH"]], F32, tag="doff")
        for t, s in [(iota_t, iota_in), (ident_t, ident_in),
                     (waug_t, waug_in), (wr_t, wr_in), (scal_t, scal_in),
                     (idx_t, idx_in), (doff_t, doff_in)]:
            nc.sync.dma_start(t[:], s.ap()[:])

        tails = {NB - 1: Nl - 128 * (NB - 1)}
        nidx_regs = {}

        def nidx_reg(n):
            if n not in nidx_regs:
                nidx_regs[n] = nc.gpsimd.to_reg(n)
            return nidx_regs[n]

        for step in range(STEP):
            # ------------------------------------------------ projection
            eng_flip = 0
            for r in range(N_CORES):
                for t in range(NB):
                    o = 128 * t
                    w = min(128, Nl - o)
                    g0 = r * Nl + o
                    xt = pools["xt"].tile([D, 128], BF16, tag="projlhs")
                    if step == 0:
                        nc.sync.dma_start(xt[:, :w], xT_in.ap()[:, g0:g0 + w])
                    else:
                        nc.sync.dma_start(
                            xt[:, :w], xT_ag.ap()[D * r:D * (r + 1), o:o + w])
                    pp = pC.tile([128, RW], F32, tag="pj")
                    nc.tensor.matmul(pp[:w, :], xt[:, :w], waug_t[:],
                                     start=True, stop=True)
                    tb = pools["tbl"].tile([128, RW], F8, tag="tbl")
                    if eng_flip % 2 == 0:
                        nc.vector.tensor_copy(tb[:w, :], pp[:w, :])
                    else:
                        nc.scalar.activation(tb[:w, :], pp[:w, :], AF.Copy)
                    eng_flip += 1
                    nc.sync.dma_start(table.ap()[g0:g0 + w, 0:RW], tb[:w, :])

            # ------------------------------------------------ gather + attn
            x_src = x_in if step == 0 else x_mid
            xt_src = xTl_in if step == 0 else xT_sh
            call_i = 0
            group_i = 0
            for isb in range(NSB):
                blocks = list(range(isb * SB, min((isb + 1) * SB, NB)))
                nb = len(blocks)
                b0 = blocks[0]
                acc = pA.tile([128, SB, 512], F32, tag="acc")
                x4 = pools["blk"].tile([128, SB, D], F32, tag="x4")
                c04 = pools["blk"].tile([128, SB, D], F32, tag="c04")
                nc.sync.dma_start(x4[:, :nb, :], x_src.ap()[:, b0:b0 + nb, :])
                nc.sync.dma_start(c04[:, :nb, :], c0_in.ap()[:, b0:b0 + nb, :])
                # x4p = (1-alpha) * x4 + c0
                x4p = pools["blk"].tile([128, SB, D], F32, tag="x4p")
                nc.vector.scalar_tensor_tensor(
                    x4p[:, :nb, :], x4[:, :nb, :], scal_t[:, 0:1],
                    c04[:, :nb, :], op0=ALU.mult, op1=ALU.add)
                er_sb = {}
                for j, b in enumerate(blocks):
                    xtb = pools["xt"].tile([D, 128], BF16, tag="erlhs")
                    nc.sync.dma_start(xtb[:], xt_src.ap()[:, 128 * b:128 * (b + 1)])
                    nc.tensor.matmul(acc[:, j, 264:264 + H], xtb[:], wr_t[:],
                                     start=True, stop=True)
                    es = pools["sm"].tile([128, H], F8, tag="erblk%d" % j)
                    nc.scalar.activation(es[:], acc[:, j, 264:264 + H], AF.Copy)
                    er_sb[b] = es

                # walk this superblock's calls/groups/chunks
                while call_i < len(meta["calls"]):
                    st, lo, nch = meta["calls"][call_i]
                    if lo >= len(meta["chunk_meta"]) or \
                       meta["chunk_meta"][lo][0] != isb:
                        break
                    n = nch * 128
                    rows = pools["rows"].tile([128, MAX_CALL, TW], F8,
                                              tag="rows")
                    icol = meta["call_cols"][call_i]
                    rows_ap = _ap(rows[:], [TW, nch], [1, TW])
                    tbl_ap = table.ap()[st * st_rows:
                                        min((st + 1) * st_rows, N), :]
                    if not SKIP_GATHER:
                        nc.gpsimd.dma_gather(
                            rows_ap, tbl_ap, idx_t[:, icol:icol + n // 16],
                            num_idxs=n, num_idxs_reg=nidx_reg(n), elem_size=TW,
                            single_packet=SINGLE_PACKET,
                            queue_num=call_i % N_QUEUES)
                    call_i += 1

                    while group_i < len(meta["groups"]):
                        gst, glo_call, g, gs = meta["groups"][group_i]
                        if glo_call != lo:
                            break
                        group_i += 1
                        cc0 = g - lo   # chunk offset within call
                        # transposed one-hot masks (host-precomputed, fp8)
                        mt = pools["mask"].tile([128, GS * 128], F8, tag="mt")
                        nc.sync.dma_start(
                            mt[:, :gs * 128],
                            mt8_in.ap()[:, g * 128:(g + gs) * 128])
                        # dst one-hot m8 per chunk (tensor_scalar, 4x mode)
                        m8 = pools["m8p"].tile([128, GS, 128], BF16, tag="m8")
                        for k in range(gs):
                            nc.vector.tensor_scalar(
                                m8[:, k, :], iota_t[:, 0:128],
                                doff_t[:, g + k:g + k + 1], None,
                                op0=ALU.is_equal)
                        # er per edge via fp8 mask matmul
                        er8 = pB.tile([128, GS * H], F32, tag="er8")
                        for k in range(gs):
                            ci = g + k
                            _, _, b = meta["chunk_meta"][ci]
                            nc.tensor.matmul(er8[:, H * k:H * (k + 1)],
                                             mt[:, 128 * k:128 * (k + 1)],
                                             er_sb[b], start=True, stop=True)
                        # t8 = er8 + el (gathered, fp8)
                        t8 = pools["sm"].tile([128, GS * H], BF16, tag="t8")
                        nc.vector.tensor_tensor(
                            t8[:, :gs * H], er8[:, :gs * H],
                            _ap(rows[:, cc0:cc0 + gs, HD:HD + H],
                                [TW, gs], [1, H]),
                            op=ALU.add)
                        lr8 = pools["sm"].tile([128, GS * H], BF16, tag="lr8")
                        nc.vector.scalar_tensor_tensor(
                            lr8[:, :gs * H], t8[:, :gs * H], NEG_SLOPE,
                            t8[:, :gs * H], op0=ALU.mult, op1=ALU.max)
                        rhs8 = pools["rhs"].tile([128, GS, RW], BF16, tag="rhs8")
                        nc.scalar.activation(
                            _ap(rhs8[:], [RW, gs], [1, H]),
                            _ap(lr8[:], [H, gs], [1, H]), AF.Exp)
                        nc.vector.tensor_tensor(
                            _ap(rhs8[:, :, H:RW], [RW, gs], [D, H], [1, D]),
                            _ap(rows[:, cc0:cc0 + gs, 0:HD],
                                [TW, gs], [D, H], [1, D]),
                            _ap(rhs8[:], [RW, gs], [1, H], [0, D]),
                            op=ALU.mult)
                        for k in range(gs):
                            ci = g + k
                            _, _, b = meta["chunk_meta"][ci]
                            j = b - b0
                            nc.tensor.matmul(
                                acc[:, j, 0:RW], m8[:, k, :], rhs8[:, k, :],
                                start=(meta["first"][(isb, b)] == ci),
                                stop=(meta["last"][(isb, b)] == ci),
                                skip_group_check=True)

                # ---- superblock epilogue (batched over blocks)
                smax = pools["sm"].tile([128, SB * H], F32, tag="smax")
                nc.vector.tensor_scalar(
                    _ap(smax[:], [H, nb], [1, H]),
                    _ap(acc[:], [512, nb], [1, H]),
                    1e-30, None, op0=ALU.max)
                srec = pools["sm"].tile([128, SB * H], F32, tag="srec")
                nc.vector.reciprocal(srec[:, :nb * H], smax[:, :nb * H])
                srec2 = pools["sm"].tile([128, SB * H], F32, tag="srec2")
                nc.vector.tensor_scalar(
                    srec2[:, :nb * H], srec[:, :nb * H], scal_t[:, 1:2], None,
                    op0=ALU.mult)
                onorm = pools["big"].tile([128, SB, H, D], F32, tag="onorm")
                nc.vector.tensor_tensor(
                    _ap(onorm[:], [H * D, nb], [D, H], [1, D]),
                    _ap(acc[:, :, H:RW], [512, nb], [D, H], [1, D]),
                    _ap(srec2[:], [H, nb], [1, H], [0, D]),
                    op=ALU.mult)
                red = pools["blk"].tile([128, SB, D], F32, tag="red")
                nc.vector.tensor_reduce(
                    _ap(red[:], [D, nb], [1, D]),
                    _ap(onorm[:], [H * D, nb], [1, D], [D, H]),
                    axis=mybir.AxisListType.X, op=ALU.add)
                xn = pools["blk"].tile([128, SB, D], F32, tag="xn")
                nc.vector.tensor_add(xn[:, :nb, :], x4p[:, :nb, :],
                                     red[:, :nb, :])
                if step < STEP - 1:
                    xnb = pools["blk"].tile([128, SB, D], BF16, tag="xnb")
                    nc.vector.tensor_copy(xnb[:, :nb, :], xn[:, :nb, :])
                    nc.sync.dma_start(x_mid.ap()[:, b0:b0 + nb, :],
                                      xn[:, :nb, :])
                    for j, b in enumerate(blocks):
                        tp = pC.tile([D, 128], BF16, tag="pj")
                        nc.tensor.transpose(tp[:], xnb[:, j, :], ident_t[:])
                        xts = pools["sm"].tile([D, 128], BF16, tag="xts")
                        nc.scalar.activation(xts[:], tp[:], AF.Copy)
                        nc.sync.dma_start(
                            xT_sh.ap()[:, 128 * b:128 * (b + 1)], xts[:])
                else:
                    for j, b in enumerate(blocks):
                        w = tails.get(b, 128)
                        nc.sync.dma_start(x_out.ap()[128 * b:128 * b + w, :],
                                          xn[:w, j, :])
            assert call_i == len(meta["calls"]) and \
                group_i == len(meta["groups"])

            if step == 0 and STEP > 1 and not SKIP_COLL:
                nc.gpsimd.collective_compute(
                    "AllGather", ALU.bypass,
                    replica_groups=[list(range(N_CORES))],
                    ins=[xT_sh.ap()[:]], outs=[xT_ag.ap()[:]])

    _split_multi_waits(nc)
    lower_extended_insts(nc)
    return nc


# ----------------------------------------------------------------------------
# entry point
# ----------------------------------------------------------------------------

def kernel(x, x0, src, dst, W, attn_l, attn_r, alpha, lamda, **kw):
    global _last_results
    x = np.asarray(x, np.float32)
    x0 = np.asarray(x0, np.float32)
    src = np.asarray(src)
    dst = np.asarray(dst)
    W = np.asarray(W, np.float32)
    attn_l = np.asarray(attn_l, np.float32)
    attn_r = np.asarray(attn_r, np.float32)
    alpha_f = float(np.asarray(alpha))
    lamda_f = float(np.asarray(lamda))

    N, D = x.shape
    H = attn_l.shape[0]
    assert N % N_CORES == 0
    meta = _plan_and_arrays(src, dst, N)
    Nl, NB = meta["Nl"], meta["NB"]
    NBP = NB * 128

    nc = _build(meta, N, D, H)

    # host-side weight prep
    W3 = W.reshape(D, H, D)
    WL = np.einsum("khd,hd->kh", W3, attn_l)
    WR = np.einsum("khd,hd->kh", W3, attn_r)
    waug = _bf(np.concatenate([W, WL], axis=1))
    wr = _bf(WR)
    iota = _bf(np.tile(np.arange(128, dtype=np.float32)[None, :], (128, 1)))
    ident = _bf(np.eye(128, dtype=np.float32))
    scal = np.zeros((128, 4), np.float32)
    scal[:, 0] = 1.0 - alpha_f
    scal[:, 1] = alpha_f / H
    c0 = (alpha_f * lamda_f) * x0

    d_idx = np.arange(128, dtype=np.float32)
    xT = _bf(x.T).copy()                      # [D, N]
    in_maps = []
    for p in range(N_CORES):
        lo = p * Nl
        xl = np.zeros((NBP, D), np.float32)
        xl[:Nl] = x[lo:lo + Nl]
        c0l = np.zeros((NBP, D), np.float32)
        c0l[:Nl] = c0[lo:lo + Nl]
        # transposed multi-chunk one-hot mask: mt8[d, ci*128+e] =
        # (dst_off(ci, e) == d), fp8 {0,1}
        mt8 = _f8(meta["doff_raw"][p][None, :] == d_idx[:, None])
        in_maps.append({
            "xT_in": np.ascontiguousarray(xT),
            "xTl_in": np.ascontiguousarray(_bf(xl.T)),
            "x_in": np.ascontiguousarray(
                xl.reshape(NB, 128, D).transpose(1, 0, 2)),
            "c0_in": np.ascontiguousarray(
                c0l.reshape(NB, 128, D).transpose(1, 0, 2)),
            "waug_in": waug, "wr_in": wr,
            "iota_in": iota, "ident_in": ident,
            "scal_in": scal,
            "idx_in": np.ascontiguousarray(
                np.tile(meta["idx_wrapped"][p], (8, 1))),
            "doff_in": np.ascontiguousarray(meta["dstoff"][p]),
            "mt8_in": np.ascontiguousarray(mt8),
        })

    trace = bool(int(os.environ.get("GAT_TRACE", "0")))
    res = run_bass_kernel_spmd(nc, in_maps, core_ids=list(range(N_CORES)),
                               trace=trace,
                               trace_cores=[0] if trace else None,
                               stitch_traces=False)
    _last_results = res
    out = np.concatenate([res.results[p]["x_out"] for p in range(N_CORES)],
                         axis=0)
    return out.astype(np.float32)


# revision 5
# speedup vs baseline: 1.3913x; 1.2230x over previous
"""GAT (graph attention) message-passing kernel for Trainium2, 8 NeuronCores.

Strategy (graph/data parallel, dst-sharded):
  - Nodes are partitioned across 8 cores by destination id (12500 each).
  - Edges are sharded by dst partition, sorted by (dst-block, src-subtable),
    and padded so every core runs an identical (SPMD) program.
  - Per step, every core projects ALL nodes (h = x @ [W | W@attn_l]) into an
    fp8 row table in its HBM ([h(256B) | el(4B) | pad] @ 512B stride), then
    indirect-gathers h[src] rows per edge (dma_gather, 4 SWDGE queues round
    robin so descriptor generation runs on all 4 Q7 core pairs), computes
    attention scores (er via host-precomputed transposed one-hot masks
    streamed from HBM as fp8 + gathered el added on DVE), and accumulates
    [softmax-denominator | weighted message sum] into per-dst-block PSUM
    with mask matmuls on TensorE.  The dst one-hot masks are built on DVE
    with per-chunk tensor_scalar(is_equal) (4x mode).
  - Block epilogue: normalize by the segment sum, head-mean, residual update.
  - Between the 2 conv steps, the updated x (transposed, bf16) is AllGathered
    across the 8 cores.
"""

import os
import math
import numpy as np
import ml_dtypes

import concourse.bass as bass
import concourse.tile as tile
import concourse.mybir as mybir
from concourse import library_config
from concourse.library_overlay import lower_extended_insts
from concourse.bass_utils import run_bass_kernel_spmd

BF16 = mybir.dt.bfloat16
F32 = mybir.dt.float32
F8 = mybir.dt.float8e4
I16 = mybir.dt.int16
AF = mybir.ActivationFunctionType
ALU = mybir.AluOpType

NEG_SLOPE = 0.2
STEP = int(os.environ.get("GAT_STEPS", "2"))
SKIP_COLL = bool(int(os.environ.get("GAT_SKIP_COLL", "0")))
SKIP_GATHER = bool(int(os.environ.get("GAT_SKIP_GATHER", "0")))
N_QUEUES = int(os.environ.get("GAT_QUEUES", "4"))
SINGLE_PACKET = bool(int(os.environ.get("GAT_SINGLE_PACKET", "0")))
N_CORES = 8
SB = 4            # blocks per superblock (PSUM accumulators alive at once)
MAX_CALL = int(os.environ.get("GAT_MAX_CALL", "32"))  # chunks per dma_gather call
GS = 8            # chunks per elementwise batch group
ST_MAX_ROWS = 25000   # subtable rows (int16 gather index limit)

_last_results = None  # BassKernelResults stash for test harness


def _bf(x):
    return np.asarray(x, np.float32).astype(ml_dtypes.bfloat16)


def _f8(x):
    return np.asarray(x, np.float32).astype(ml_dtypes.float8_e4m3fn)


# ----------------------------------------------------------------------------
# host-side preprocessing
# ----------------------------------------------------------------------------

def _plan_and_arrays(src, dst, N):
    """Shard/sort/pad edges; build the shared chunk plan and per-core arrays."""
    Nl = N // N_CORES
    NB = (Nl + 127) // 128
    NSB = (NB + SB - 1) // SB
    NST = max(1, math.ceil(N / ST_MAX_ROWS))
    st_rows = math.ceil(N / NST)

    core = dst // Nl
    percore = []
    for p in range(N_CORES):
        sel = np.nonzero(core == p)[0]
        s = src[sel].astype(np.int64)
        d = (dst[sel] - p * Nl).astype(np.int64)
        blk = d >> 7
        st = s // st_rows
        order = np.lexsort((s, st, blk))
        percore.append((s[order], d[order], blk[order], st[order]))

    counts = np.zeros((N_CORES, NB, NST), np.int64)
    for p in range(N_CORES):
        _, _, blk, st = percore[p]
        np.add.at(counts, (p, blk, st), 1)
    nchunks = (counts.max(axis=0) + 127) // 128          # [NB, NST]

    # canonical chunk emission order
    chunk_meta = []   # (isb, st, b) per chunk
    calls = []        # (st, chunk_lo, n_chunks)
    for isb in range(NSB):
        blocks = range(isb * SB, min((isb + 1) * SB, NB))
        for st in range(NST):
            run_lo = len(chunk_meta)
            for b in blocks:
                for _ in range(int(nchunks[b, st])):
                    chunk_meta.append((isb, st, b))
            n = len(chunk_meta) - run_lo
            o = run_lo
            while n > 0:
                take = min(n, MAX_CALL)
                calls.append((st, o, take))
                o += take
                n -= take
    NCH = len(chunk_meta)

    # first/last chunk index per (isb, b) for PSUM start/stop flags
    first = {}
    last = {}
    for ci, (isb, st, b) in enumerate(chunk_meta):
        key = (isb, b)
        if key not in first:
            first[key] = ci
        last[key] = ci

    # per-core edge arrays in padded chunk order
    idx_all = np.zeros((N_CORES, NCH * 128), np.int16)
    doff_all = np.full((N_CORES, NCH * 128), 255.0, np.float32)
    for p in range(N_CORES):
        s, d, blk, st = percore[p]
        # build run boundaries of the (blk, st)-sorted edge list
        runs = {}
        i = 0
        M = len(s)
        while i < M:
            k = (blk[i], st[i])
            j = i
            while j < M and blk[j] == k[0] and st[j] == k[1]:
                j += 1
            runs[k] = (i, j)
            i = j
        cursor = {k: v[0] for k, v in runs.items()}
        for ci, (isb, t, b) in enumerate(chunk_meta):
            base = ci * 128
            k = (b, t)
            if k in runs:
                lo = cursor[k]
                hi = min(lo + 128, runs[k][1])
                n = hi - lo
                cursor[k] = hi
                if n > 0:
                    idx_all[p, base:base + n] = (s[lo:hi] - t * st_rows).astype(np.int16)
                    doff_all[p, base:base + n] = (d[lo:hi] - b * 128).astype(np.float32)
        for k, (lo, hi) in runs.items():
            assert cursor[k] == hi, "edge run not fully consumed"

    # gather-call wrapped idx layout: per call [16, n/16], concat on free axis
    idxw_cols = NCH * 8
    idx_wrapped = np.zeros((N_CORES, 16, idxw_cols), np.int16)
    col = 0
    call_cols = []
    for (t, lo, nch) in calls:
        n = nch * 128
        for p in range(N_CORES):
            seg = idx_all[p, lo * 128: lo * 128 + n]
            idx_wrapped[p, :, col:col + n // 16] = seg.reshape(-1, 16).T
        call_cols.append(col)
        col += n // 16
    assert col == idxw_cols

    # dstoff [128, NCH]: partition = edge-in-chunk
    doff = doff_all.reshape(N_CORES, NCH, 128).transpose(0, 2, 1)

    groups = []
    for (t, lo, nch) in calls:
        g = lo
        while g < lo + nch:
            take = min(GS, lo + nch - g)
            groups.append((t, lo, g, take))  # (st, call_lo, group_lo, size)
            g += take

    return dict(Nl=Nl, NB=NB, NSB=NSB, NST=NST, st_rows=st_rows, NCH=NCH,
                chunk_meta=chunk_meta, calls=calls, call_cols=call_cols,
                groups=groups, first=first, last=last,
                idx_wrapped=idx_wrapped, dstoff=doff, doff_raw=doff_all,
                idxw_cols=idxw_cols)


# ----------------------------------------------------------------------------
# device program
# ----------------------------------------------------------------------------

def _split_multi_waits(nc):
    """walrus codegen only accepts one sync-wait per instruction; hoist any
    extra waits onto same-engine NOPs inserted right before the instruction."""
    n_id = 0
    for f in nc.m.functions:
        for blk in f.blocks:
            out = []
            for ins in blk.instructions:
                si = ins.sync_info
                if si is not None and len(si.on_wait) > 1 \
                        and ins.engine is not None:
                    waits = list(si.on_wait)
                    for w in waits[:-1]:
                        nop = mybir.InstNoOp(name=f"I-wsplit-{n_id}", ins=[],
                                             outs=[])
                        n_id += 1
                        nop.engine = ins.engine
                        nop.sync_info = mybir.SyncInfo(on_wait=[w],
                                                       on_update=[])
                        nc.inst_map[nop.name] = nop
                        out.append(nop)
                    ins.sync_info = mybir.SyncInfo(on_wait=[waits[-1]],
                                                   on_update=list(si.on_update))
                out.append(ins)
            blk.instructions = out

def _ap(base, *dims):
    """Rebuild AP with the same tensor/offset/partition dim, custom free dims."""
    return bass.AP(base.tensor, base.offset,
                   [list(base.ap[0])] + [list(d) for d in dims])


def _build(meta, N, D, H):
    Nl, NB, NSB, NST = meta["Nl"], meta["NB"], meta["NSB"], meta["NST"]
    st_rows = meta["st_rows"]
    NBP = NB * 128
    HD = H * D            # 256
    RW = HD + H           # 260 elems (h | el), fp8 -> 260B used
    TW = 512              # fp8 table row stride: 512B (gather elem size)

    nc = bass.Bass("TRN2", target_bir_lowering=False, debug=False,
                   enable_asserts=False, num_devices=N_CORES,
                   num_swdge_queues=N_QUEUES)

    # ---- DRAM tensors
    xT_in = nc.dram_tensor("xT_in", [D, N], BF16, kind="ExternalInput")
    xTl_in = nc.dram_tensor("xTl_in", [D, NBP], BF16, kind="ExternalInput")
    x_in = nc.dram_tensor("x_in", [128, NB, D], F32, kind="ExternalInput")
    c0_in = nc.dram_tensor("c0_in", [128, NB, D], F32, kind="ExternalInput")
    waug_in = nc.dram_tensor("waug_in", [D, RW], BF16, kind="ExternalInput")
    wr_in = nc.dram_tensor("wr_in", [D, H], BF16, kind="ExternalInput")
    iota_in = nc.dram_tensor("iota_in", [128, 128], BF16, kind="ExternalInput")
    ident_in = nc.dram_tensor("ident_in", [128, 128], BF16, kind="ExternalInput")
    scal_in = nc.dram_tensor("scal_in", [128, 4], F32, kind="ExternalInput")
    idx_in = nc.dram_tensor("idx_in", [128, meta["idxw_cols"]], I16,
                            kind="ExternalInput")
    doff_in = nc.dram_tensor("doff_in", [128, meta["NCH"]], BF16,
                             kind="ExternalInput")
    mt8_in = nc.dram_tensor("mt8_in", [128, meta["NCH"] * 128], F8,
                            kind="ExternalInput")

    table = nc.dram_tensor("table", [N, TW], F8, kind="Internal")
    x_mid = nc.dram_tensor("x_mid", [128, NB, D], F32, kind="Internal")
    xT_sh = nc.dram_tensor("xT_sh", [D, NBP], BF16, kind="Internal")
    xT_ag = nc.dram_tensor("xT_ag", [D * N_CORES, NBP], BF16, kind="Internal",
                           addr_space="Shared")
    x_out = nc.dram_tensor("x_out", [Nl, D], F32, kind="ExternalOutput")

    from contextlib import ExitStack
    with tile.TileContext(nc) as tc, ExitStack() as es_:
        nc.gpsimd.load_library(library_config.mlp)
        cp = es_.enter_context(tc.tile_pool(name="consts", bufs=1))
        pools = {}
        for nm, bufs in [("xt", 8), ("rows", 4), ("mask", 3), ("m8p", 2),
                         ("rhs", 3), ("sm", 4), ("tbl", 8), ("blk", 3),
                         ("big", 2)]:
            pools[nm] = es_.enter_context(tc.tile_pool(name=nm, bufs=bufs))
        pA = es_.enter_context(tc.tile_pool(name="pacc", bufs=1, space="PSUM"))
        pB = es_.enter_context(tc.tile_pool(name="per8", bufs=2, space="PSUM"))
        pC = es_.enter_context(tc.tile_pool(name="ppj", bufs=2, space="PSUM"))

        # ---- load constants
        iota_t = cp.tile([128, 128], BF16, tag="iota")
        ident_t = cp.tile([128, 128], BF16, tag="ident")
        waug_t = cp.tile([D, RW], BF16, tag="waug")
        wr_t = cp.tile([D, H], BF16, tag="wr")
        scal_t = cp.tile([128, 4], F32, tag="scal")
        idx_t = cp.tile([128, meta["idxw_cols"]], I16, tag="idx")
        doff_t = cp.tile([128, meta["NCH"]], BF16, tag="doff")
        for t, s in [(iota_t, iota_in), (ident_t, ident_in),
                     (waug_t, waug_in), (wr_t, wr_in), (scal_t, scal_in),
                     (idx_t, idx_in), (doff_t, doff_in)]:
            nc.sync.dma_start(t[:], s.ap()[:])

        tails = {NB - 1: Nl - 128 * (NB - 1)}
        nidx_regs = {}

        def nidx_reg(n):
            if n not in nidx_regs:
                nidx_regs[n] = nc.gpsimd.to_reg(n)
            return nidx_regs[n]

        for step in range(STEP):
            # ------------------------------------------------ projection
            eng_flip = 0
            for r in range(N_CORES):
                for t in range(NB):
                    o = 128 * t
                    w = min(128, Nl - o)
                    g0 = r * Nl + o
                    xt = pools["xt"].tile([D, 128], BF16, tag="projlhs")
                    if step == 0:
                        nc.sync.dma_start(xt[:, :w], xT_in.ap()[:, g0:g0 + w])
                    else:
                        nc.sync.dma_start(
                            xt[:, :w], xT_ag.ap()[D * r:D * (r + 1), o:o + w])
                    pp = pC.tile([128, RW], F32, tag="pj")
                    nc.tensor.matmul(pp[:w, :], xt[:, :w], waug_t[:],
                                     start=True, stop=True)
                    tb = pools["tbl"].tile([128, RW], F8, tag="tbl")
                    if eng_flip % 2 == 0:
                        nc.vector.tensor_copy(tb[:w, :], pp[:w, :])
                    else:
                        nc.scalar.activation(tb[:w, :], pp[:w, :], AF.Copy)
                    eng_flip += 1
                    nc.sync.dma_start(table.ap()[g0:g0 + w, 0:RW], tb[:w, :])

            # ------------------------------------------------ gather + attn
            x_src = x_in if step == 0 else x_mid
            xt_src = xTl_in if step == 0 else xT_sh
            call_i = 0
            group_i = 0
            for isb in range(NSB):
                blocks = list(range(isb * SB, min((isb + 1) * SB, NB)))
                nb = len(blocks)
                b0 = blocks[0]
                acc = pA.tile([128, SB, 512], F32, tag="acc")
                x4 = pools["blk"].tile([128, SB, D], F32, tag="x4")
                c04 = pools["blk"].tile([128, SB, D], F32, tag="c04")
                nc.sync.dma_start(x4[:, :nb, :], x_src.ap()[:, b0:b0 + nb, :])
                nc.sync.dma_start(c04[:, :nb, :], c0_in.ap()[:, b0:b0 + nb, :])
                # x4p = (1-alpha) * x4 + c0
                x4p = pools["blk"].tile([128, SB, D], F32, tag="x4p")
                nc.vector.scalar_tensor_tensor(
                    x4p[:, :nb, :], x4[:, :nb, :], scal_t[:, 0:1],
                    c04[:, :nb, :], op0=ALU.mult, op1=ALU.add)
                er_sb = {}
                for j, b in enumerate(blocks):
                    xtb = pools["xt"].tile([D, 128], BF16, tag="erlhs")
                    nc.sync.dma_start(xtb[:], xt_src.ap()[:, 128 * b:128 * (b + 1)])
                    nc.tensor.matmul(acc[:, j, 264:264 + H], xtb[:], wr_t[:],
                                     start=True, stop=True)
                    es = pools["sm"].tile([128, H], F8, tag="erblk%d" % j)
                    nc.scalar.activation(es[:], acc[:, j, 264:264 + H], AF.Copy)
                    er_sb[b] = es

                # walk this superblock's calls/groups/chunks
                while call_i < len(meta["calls"]):
                    st, lo, nch = meta["calls"][call_i]
                    if lo >= len(meta["chunk_meta"]) or \
                       meta["chunk_meta"][lo][0] != isb:
                        break
                    n = nch * 128
                    rows = pools["rows"].tile([128, MAX_CALL, TW], F8,
                                              tag="rows")
                    icol = meta["call_cols"][call_i]
                    rows_ap = _ap(rows[:], [TW, nch], [1, TW])
                    tbl_ap = table.ap()[st * st_rows:
                                        min((st + 1) * st_rows, N), :]
                    if not SKIP_GATHER:
                        nc.gpsimd.dma_gather(
                            rows_ap, tbl_ap, idx_t[:, icol:icol + n // 16],
                            num_idxs=n, num_idxs_reg=nidx_reg(n), elem_size=TW,
                            single_packet=SINGLE_PACKET,
                            queue_num=call_i % N_QUEUES)
                    call_i += 1

                    while group_i < len(meta["groups"]):
                        gst, glo_call, g, gs = meta["groups"][group_i]
                        if glo_call != lo:
                            break
                        group_i += 1
                        cc0 = g - lo   # chunk offset within call
                        # transposed one-hot masks (host-precomputed, fp8)
                        mt = pools["mask"].tile([128, GS * 128], F8, tag="mt")
                        nc.sync.dma_start(
                            mt[:, :gs * 128],
                            mt8_in.ap()[:, g * 128:(g + gs) * 128])
                        # dst one-hot m8 (grouped is_equal)
                        m8 = pools["m8p"].tile([128, GS, 128], BF16, tag="m8")
                        nc.vector.tensor_tensor(
                            _ap(m8[:], [128, gs], [1, 128]),
                            _ap(iota_t[:], [0, gs], [1, 128]),
                            _ap(doff_t[:, g:g + gs], [1, gs], [0, 128]),
                            op=ALU.is_equal)
                        # er per edge via fp8 mask matmul
                        er8 = pB.tile([128, GS * H], F32, tag="er8")
                        for k in range(gs):
                            ci = g + k
                            _, _, b = meta["chunk_meta"][ci]
                            nc.tensor.matmul(er8[:, H * k:H * (k + 1)],
                                             mt[:, 128 * k:128 * (k + 1)],
                                             er_sb[b], start=True, stop=True)
                        # t8 = er8 + el (gathered, fp8)
                        t8 = pools["sm"].tile([128, GS * H], BF16, tag="t8")
                        nc.vector.tensor_tensor(
                            t8[:, :gs * H], er8[:, :gs * H],
                            _ap(rows[:, cc0:cc0 + gs, HD:HD + H],
                                [TW, gs], [1, H]),
                            op=ALU.add)
                        lr8 = pools["sm"].tile([128, GS * H], BF16, tag="lr8")
                        nc.vector.scalar_tensor_tensor(
                            lr8[:, :gs * H], t8[:, :gs * H], NEG_SLOPE,
                            t8[:, :gs * H], op0=ALU.mult, op1=ALU.max)
                        rhs8 = pools["rhs"].tile([128, GS, RW], BF16, tag="rhs8")
                        nc.scalar.activation(
                            _ap(rhs8[:], [RW, gs], [1, H]),
                            _ap(lr8[:], [H, gs], [1, H]), AF.Exp)
                        nc.vector.tensor_tensor(
                            _ap(rhs8[:, :, H:RW], [RW, gs], [D, H], [1, D]),
                            _ap(rows[:, cc0:cc0 + gs, 0:HD],
                                [TW, gs], [D, H], [1, D]),
                            _ap(rhs8[:], [RW, gs], [1, H], [0, D]),
                            op=ALU.mult)
                        for k in range(gs):
                            ci = g + k
                            _, _, b = meta["chunk_meta"][ci]
                            j = b - b0
                            nc.tensor.matmul(
                                acc[:, j, 0:RW], m8[:, k, :], rhs8[:, k, :],
                                start=(meta["first"][(isb, b)] == ci),
                                stop=(meta["last"][(isb, b)] == ci),
                                skip_group_check=True)

                # ---- superblock epilogue (batched over blocks)
                smax = pools["sm"].tile([128, SB * H], F32, tag="smax")
                nc.vector.tensor_scalar(
                    _ap(smax[:], [H, nb], [1, H]),
                    _ap(acc[:], [512, nb], [1, H]),
                    1e-30, None, op0=ALU.max)
                srec = pools["sm"].tile([128, SB * H], F32, tag="srec")
                nc.vector.reciprocal(srec[:, :nb * H], smax[:, :nb * H])
                srec2 = pools["sm"].tile([128, SB * H], F32, tag="srec2")
                nc.vector.tensor_scalar(
                    srec2[:, :nb * H], srec[:, :nb * H], scal_t[:, 1:2], None,
                    op0=ALU.mult)
                onorm = pools["big"].tile([128, SB, H, D], F32, tag="onorm")
                nc.vector.tensor_tensor(
                    _ap(onorm[:], [H * D, nb], [D, H], [1, D]),
                    _ap(acc[:, :, H:RW], [512, nb], [D, H], [1, D]),
                    _ap(srec2[:], [H, nb], [1, H], [0, D]),
                    op=ALU.mult)
                red = pools["blk"].tile([128, SB, D], F32, tag="red")
                nc.vector.tensor_reduce(
                    _ap(red[:], [D, nb], [1, D]),
                    _ap(onorm[:], [H * D, nb], [1, D], [D, H]),
                    axis=mybir.AxisListType.X, op=ALU.add)
                xn = pools["blk"].tile([128, SB, D], F32, tag="xn")
                nc.vector.tensor_add(xn[:, :nb, :], x4p[:, :nb, :],
                                     red[:, :nb, :])
                if step < STEP - 1:
                    xnb = pools["blk"].tile([128, SB, D], BF16, tag="xnb")
                    nc.vector.tensor_copy(xnb[:, :nb, :], xn[:, :nb, :])
                    nc.sync.dma_start(x_mid.ap()[:, b0:b0 + nb, :],
                                      xn[:, :nb, :])
                    for j, b in enumerate(blocks):
                        tp = pC.tile([D, 128], BF16, tag="pj")
                        nc.tensor.transpose(tp[:], xnb[:, j, :], ident_t[:])
                        xts = pools["sm"].tile([D, 128], BF16, tag="xts")
                        nc.scalar.activation(xts[:], tp[:], AF.Copy)
                        nc.sync.dma_start(
                            xT_sh.ap()[:, 128 * b:128 * (b + 1)], xts[:])
                else:
                    for j, b in enumerate(blocks):
                        w = tails.get(b, 128)
                        nc.sync.dma_start(x_out.ap()[128 * b:128 * b + w, :],
                                          xn[:w, j, :])
            assert call_i == len(meta["calls"]) and \
                group_i == len(meta["groups"])

            if step == 0 and STEP > 1 and not SKIP_COLL:
                nc.gpsimd.collective_compute(
                    "AllGather", ALU.bypass,
                    replica_groups=[list(range(N_CORES))],
                    ins=[xT_sh.ap()[:]], outs=[xT_ag.ap()[:]])

    _split_multi_waits(nc)
    lower_extended_insts(nc)
    return nc


# ----------------------------------------------------------------------------
# entry point
# ----------------------------------------------------------------------------

def kernel(x, x0, src, dst, W, attn_l, attn_r, alpha, lamda, **kw):
    global _last_results
    x = np.asarray(x, np.float32)
    x0 = np.asarray(x0, np.float32)
    src = np.asarray(src)
    dst = np.asarray(dst)
    W = np.asarray(W, np.float32)
    attn_l = np.asarray(attn_l, np.float32)
    attn_r = np.asarray(attn_r, np.float32)
    alpha_f = float(np.asarray(alpha))
    lamda_f = float(np.asarray(lamda))

    N, D = x.shape
    H = attn_l.shape[0]
    assert N % N_CORES == 0
    meta = _plan_and_arrays(src, dst, N)
    Nl, NB = meta["Nl"], meta["NB"]
    NBP = NB * 128

    nc = _build(meta, N, D, H)

    # host-side weight prep
    W3 = W.reshape(D, H, D)
    WL = np.einsum("khd,hd->kh", W3, attn_l)
    WR = np.einsum("khd,hd->kh", W3, attn_r)
    waug = _bf(np.concatenate([W, WL], axis=1))
    wr = _bf(WR)
    iota = _bf(np.tile(np.arange(128, dtype=np.float32)[None, :], (128, 1)))
    ident = _bf(np.eye(128, dtype=np.float32))
    scal = np.zeros((128, 4), np.float32)
    scal[:, 0] = 1.0 - alpha_f
    scal[:, 1] = alpha_f / H
    c0 = (alpha_f * lamda_f) * x0

    d_idx = np.arange(128, dtype=np.float32)
    xT = _bf(x.T).copy()                      # [D, N]
    in_maps = []
    for p in range(N_CORES):
        lo = p * Nl
        xl = np.zeros((NBP, D), np.float32)
        xl[:Nl] = x[lo:lo + Nl]
        c0l = np.zeros((NBP, D), np.float32)
        c0l[:Nl] = c0[lo:lo + Nl]
        # transposed multi-chunk one-hot mask: mt8[d, ci*128+e] =
        # (dst_off(ci, e) == d), fp8 {0,1}
        mt8 = _f8(meta["doff_raw"][p][None, :] == d_idx[:, None])
        in_maps.append({
            "xT_in": np.ascontiguousarray(xT),
            "xTl_in": np.ascontiguousarray(_bf(xl.T)),
            "x_in": np.ascontiguousarray(
                xl.reshape(NB, 128, D).transpose(1, 0, 2)),
            "c0_in": np.ascontiguousarray(
                c0l.reshape(NB, 128, D).transpose(1, 0, 2)),
            "waug_in": waug, "wr_in": wr,
            "iota_in": iota, "ident_in": ident,
            "scal_in": scal,
            "idx_in": np.ascontiguousarray(
                np.tile(meta["idx_wrapped"][p], (8, 1))),
            "doff_in": np.ascontiguousarray(_bf(meta["dstoff"][p])),
            "mt8_in": np.ascontiguousarray(mt8),
        })

    trace = bool(int(os.environ.get("GAT_TRACE", "0")))
    res = run_bass_kernel_spmd(nc, in_maps, core_ids=list(range(N_CORES)),
                               trace=trace,
                               trace_cores=[0] if trace else None,
                               stitch_traces=False)
    _last_results = res
    out = np.concatenate([res.results[p]["x_out"] for p in range(N_CORES)],
                         axis=0)
    return out.astype(np.float32)


# revision 14
# speedup vs baseline: 1.4168x; 1.0183x over previous
"""GAT (graph attention) message-passing kernel for Trainium2, 8 NeuronCores.

Strategy (graph/data parallel, dst-sharded):
  - Nodes are partitioned across 8 cores by destination id (12500 each).
  - Edges are sharded by dst partition, sorted by (dst-block, src-subtable),
    and padded so every core runs an identical (SPMD) program.
  - Per step, every core projects ALL nodes (h = x @ [W | W@attn_l]) into an
    fp8 row table in its HBM ([h(256B) | el(4B) | pad] @ 512B stride), then
    indirect-gathers h[src] rows per edge (dma_gather, 4 SWDGE queues round
    robin so descriptor generation runs on all 4 Q7 core pairs), computes
    attention scores (er via host-precomputed transposed one-hot masks
    streamed from HBM as fp8 + gathered el added on DVE), and accumulates
    [softmax-denominator | weighted message sum] into per-dst-block PSUM
    with mask matmuls on TensorE.  The dst one-hot masks are built on DVE
    with per-chunk tensor_scalar(is_equal) (4x mode).
  - Block epilogue: normalize by the segment sum, head-mean, residual update.
  - Between the 2 conv steps, the updated x (transposed, bf16) is AllGathered
    across the 8 cores.
"""

import os
import math
import numpy as np
import ml_dtypes

import concourse.bass as bass
import concourse.tile as tile
import concourse.mybir as mybir
from concourse import library_config
from concourse.library_overlay import lower_extended_insts
from concourse.bass_utils import run_bass_kernel_spmd

BF16 = mybir.dt.bfloat16
F32 = mybir.dt.float32
F8 = mybir.dt.float8e4
I16 = mybir.dt.int16
AF = mybir.ActivationFunctionType
ALU = mybir.AluOpType

NEG_SLOPE = 0.2
STEP = int(os.environ.get("GAT_STEPS", "2"))
SKIP_COLL = bool(int(os.environ.get("GAT_SKIP_COLL", "0")))
SKIP_GATHER = bool(int(os.environ.get("GAT_SKIP_GATHER", "0")))
N_QUEUES = int(os.environ.get("GAT_QUEUES", "4"))
SINGLE_PACKET = bool(int(os.environ.get("GAT_SINGLE_PACKET", "0")))
N_CORES = 8
SB = 4            # blocks per superblock (PSUM accumulators alive at once)
MAX_CALL = int(os.environ.get("GAT_MAX_CALL", "8"))  # chunks per dma_gather call
GS = 8            # chunks per elementwise batch group
ST_MAX_ROWS = 25000   # subtable rows (int16 gather index limit)
PAD_IDX = int(os.environ.get("GAT_PAD_IDX", "0"))

_last_results = None  # BassKernelResults stash for test harness


def _bf(x):
    return np.asarray(x, np.float32).astype(ml_dtypes.bfloat16)


def _f8(x):
    return np.asarray(x, np.float32).astype(ml_dtypes.float8_e4m3fn)


# ----------------------------------------------------------------------------
# host-side preprocessing
# ----------------------------------------------------------------------------

def _plan_and_arrays(src, dst, N):
    """Shard/sort/pad edges; build the shared chunk plan and per-core arrays."""
    Nl = N // N_CORES
    NB = (Nl + 127) // 128
    NSB = (NB + SB - 1) // SB
    NST = max(1, math.ceil(N / ST_MAX_ROWS))
    st_rows = math.ceil(N / NST)

    core = dst // Nl
    percore = []
    for p in range(N_CORES):
        sel = np.nonzero(core == p)[0]
        s = src[sel].astype(np.int64)
        d = (dst[sel] - p * Nl).astype(np.int64)
        blk = d >> 7
        st = s // st_rows
        order = np.lexsort((s, st, blk))
        percore.append((s[order], d[order], blk[order], st[order]))

    counts = np.zeros((N_CORES, NB, NST), np.int64)
    for p in range(N_CORES):
        _, _, blk, st = percore[p]
        np.add.at(counts, (p, blk, st), 1)
    nchunks = (counts.max(axis=0) + 127) // 128          # [NB, NST]

    # canonical chunk emission order.  One call per (b, st) run so that all
    # idx padding is TRAILING within its call: the gather ucode trims
    # trailing negative idxs before descriptor generation, skipping the
    # padding's Pool + DMA cost entirely.
    chunk_meta = []   # (isb, st, b) per chunk
    calls = []        # (st, chunk_lo, n_chunks)
    for isb in range(NSB):
        blocks = range(isb * SB, min((isb + 1) * SB, NB))
        for st in range(NST):
            for b in blocks:
                run_lo = len(chunk_meta)
                for _ in range(int(nchunks[b, st])):
                    chunk_meta.append((isb, st, b))
                n = len(chunk_meta) - run_lo
                o = run_lo
                while n > 0:
                    take = min(n, MAX_CALL)
                    calls.append((st, o, take))
                    o += take
                    n -= take
    NCH = len(chunk_meta)

    # first/last chunk index per (isb, b) for PSUM start/stop flags
    first = {}
    last = {}
    for ci, (isb, st, b) in enumerate(chunk_meta):
        key = (isb, b)
        if key not in first:
            first[key] = ci
        last[key] = ci

    # per-core edge arrays in padded chunk order.  NOTE: idx padding must be
    # >= 0 (a valid row; masks zero its contribution).  Negative padding
    # desyncs the gather's decode-side ring reservation (sized from the
    # num_idxs register) from the Q7 impl's trimmed descriptor count,
    # leaving stale descriptors that hang the DMA engines.
    idx_all = np.full((N_CORES, NCH * 128), PAD_IDX, np.int16)
    doff_all = np.full((N_CORES, NCH * 128), 255.0, np.float32)
    for p in range(N_CORES):
        s, d, blk, st = percore[p]
        # build run boundaries of the (blk, st)-sorted edge list
        runs = {}
        i = 0
        M = len(s)
        while i < M:
            k = (blk[i], st[i])
            j = i
            while j < M and blk[j] == k[0] and st[j] == k[1]:
                j += 1
            runs[k] = (i, j)
            i = j
        cursor = {k: v[0] for k, v in runs.items()}
        for ci, (isb, t, b) in enumerate(chunk_meta):
            base = ci * 128
            k = (b, t)
            if k in runs:
                lo = cursor[k]
                hi = min(lo + 128, runs[k][1])
                n = hi - lo
                cursor[k] = hi
                if n > 0:
                    idx_all[p, base:base + n] = (s[lo:hi] - t * st_rows).astype(np.int16)
                    doff_all[p, base:base + n] = (d[lo:hi] - b * 128).astype(np.float32)
        for k, (lo, hi) in runs.items():
            assert cursor[k] == hi, "edge run not fully consumed"

    # gather-call wrapped idx layout: per call [16, n/16], concat on free axis
    idxw_cols = NCH * 8
    idx_wrapped = np.zeros((N_CORES, 16, idxw_cols), np.int16)
    col = 0
    call_cols = []
    for (t, lo, nch) in calls:
        n = nch * 128
        for p in range(N_CORES):
            seg = idx_all[p, lo * 128: lo * 128 + n]
            idx_wrapped[p, :, col:col + n // 16] = seg.reshape(-1, 16).T
        call_cols.append(col)
        col += n // 16
    assert col == idxw_cols

    # dstoff [128, NCH]: partition = edge-in-chunk
    doff = doff_all.reshape(N_CORES, NCH, 128).transpose(0, 2, 1)

    groups = []
    for (t, lo, nch) in calls:
        g = lo
        while g < lo + nch:
            take = min(GS, lo + nch - g)
            groups.append((t, lo, g, take))  # (st, call_lo, group_lo, size)
            g += take

    return dict(Nl=Nl, NB=NB, NSB=NSB, NST=NST, st_rows=st_rows, NCH=NCH,
                chunk_meta=chunk_meta, calls=calls, call_cols=call_cols,
                groups=groups, first=first, last=last,
                idx_wrapped=idx_wrapped, dstoff=doff, doff_raw=doff_all,
                idxw_cols=idxw_cols)


# ----------------------------------------------------------------------------
# device program
# ----------------------------------------------------------------------------

def _split_multi_waits(nc):
    """walrus codegen only accepts one sync-wait per instruction; hoist any
    extra waits onto same-engine NOPs inserted right before the instruction."""
    n_id = 0
    for f in nc.m.functions:
        for blk in f.blocks:
            out = []
            for ins in blk.instructions:
                si = ins.sync_info
                if si is not None and len(si.on_wait) > 1 \
                        and ins.engine is not None:
                    waits = list(si.on_wait)
                    for w in waits[:-1]:
                        nop = mybir.InstNoOp(name=f"I-wsplit-{n_id}", ins=[],
                                             outs=[])
                        n_id += 1
                        nop.engine = ins.engine
                        nop.sync_info = mybir.SyncInfo(on_wait=[w],
                                                       on_update=[])
                        nc.inst_map[nop.name] = nop
                        out.append(nop)
                    ins.sync_info = mybir.SyncInfo(on_wait=[waits[-1]],
                                                   on_update=list(si.on_update))
                out.append(ins)
            blk.instructions = out

def _ap(base, *dims):
    """Rebuild AP with the same tensor/offset/partition dim, custom free dims."""
    return bass.AP(base.tensor, base.offset,
                   [list(base.ap[0])] + [list(d) for d in dims])


def _build(meta, N, D, H):
    Nl, NB, NSB, NST = meta["Nl"], meta["NB"], meta["NSB"], meta["NST"]
    st_rows = meta["st_rows"]
    NBP = NB * 128
    HD = H * D            # 256
    RW = HD + H           # 260 elems (h | el), fp8 -> 260B used
    TW = 512              # fp8 table row stride: 512B (gather elem size)

    nc = bass.Bass("TRN2", target_bir_lowering=False, debug=False,
                   enable_asserts=False, num_devices=N_CORES,
                   num_swdge_queues=N_QUEUES)

    # ---- DRAM tensors
    xT_in = nc.dram_tensor("xT_in", [D, N], BF16, kind="ExternalInput")
    xTl_in = nc.dram_tensor("xTl_in", [D, NBP], BF16, kind="ExternalInput")
    x_in = nc.dram_tensor("x_in", [128, NB, D], F32, kind="ExternalInput")
    c0_in = nc.dram_tensor("c0_in", [128, NB, D], F32, kind="ExternalInput")
    waug_in = nc.dram_tensor("waug_in", [D, RW], BF16, kind="ExternalInput")
    wr_in = nc.dram_tensor("wr_in", [D, H], BF16, kind="ExternalInput")
    iota_in = nc.dram_tensor("iota_in", [128, 128], BF16, kind="ExternalInput")
    ident_in = nc.dram_tensor("ident_in", [128, 128], BF16, kind="ExternalInput")
    scal_in = nc.dram_tensor("scal_in", [128, 4], F32, kind="ExternalInput")
    idx_in = nc.dram_tensor("idx_in", [128, meta["idxw_cols"]], I16,
                            kind="ExternalInput")
    doff_in = nc.dram_tensor("doff_in", [128, meta["NCH"]], BF16,
                             kind="ExternalInput")
    mt8_in = nc.dram_tensor("mt8_in", [128, meta["NCH"] * 128], F8,
                            kind="ExternalInput")

    # double-buffered row table: step-1 projection writes overlap step-0
    # gather reads, so each step gathers from its own table
    tables = [nc.dram_tensor("table%d" % s, [N, TW], F8, kind="Internal")
              for s in range(STEP)]
    x_mid = nc.dram_tensor("x_mid", [128, NB, D], F32, kind="Internal")
    # per-superblock xT shards + AllGather outputs (chunked collective so
    # step-1 projection can start as soon as each superblock's AG lands)
    sb_cols = [128 * (min((k + 1) * SB, NB) - k * SB) for k in range(NSB)]
    xT_sh = [nc.dram_tensor("xT_sh%d" % k, [D, sb_cols[k]], BF16,
                            kind="Internal") for k in range(NSB)]
    xT_ag = [nc.dram_tensor("xT_ag%d" % k, [D * N_CORES, sb_cols[k]], BF16,
                            kind="Internal", addr_space="Shared")
             for k in range(NSB)]
    x_out = nc.dram_tensor("x_out", [Nl, D], F32, kind="ExternalOutput")

    from contextlib import ExitStack
    with tile.TileContext(nc) as tc, ExitStack() as es_:
        nc.gpsimd.load_library(library_config.mlp)
        cp = es_.enter_context(tc.tile_pool(name="consts", bufs=1))
        pools = {}
        for nm, bufs in [("xt", 8), ("rows", 4), ("mask", 3), ("m8p", 2),
                         ("rhs", 3), ("sm", 4), ("tbl", 8), ("blk", 3),
                         ("big", 2)]:
            pools[nm] = es_.enter_context(tc.tile_pool(name=nm, bufs=bufs))
        pA = es_.enter_context(tc.tile_pool(name="pacc", bufs=1, space="PSUM"))
        pB = es_.enter_context(tc.tile_pool(name="per8", bufs=2, space="PSUM"))
        pC = es_.enter_context(tc.tile_pool(name="ppj", bufs=2, space="PSUM"))

        # ---- load constants
        iota_t = cp.tile([128, 128], BF16, tag="iota")
        ident_t = cp.tile([128, 128], BF16, tag="ident")
        waug_t = cp.tile([D, RW], BF16, tag="waug")
        wr_t = cp.tile([D, H], BF16, tag="wr")
        scal_t = cp.tile([128, 4], F32, tag="scal")
        idx_t = cp.tile([128, meta["idxw_cols"]], I16, tag="idx")
        doff_t = cp.tile([128, meta["NCH"]], BF16, tag="doff")
        for t, s in [(iota_t, iota_in), (ident_t, ident_in),
                     (waug_t, waug_in), (wr_t, wr_in), (scal_t, scal_in),
                     (idx_t, idx_in), (doff_t, doff_in)]:
            nc.sync.dma_start(t[:], s.ap()[:])

        tails = {NB - 1: Nl - 128 * (NB - 1)}
        nidx_regs = {}

        def nidx_reg(n):
            if n not in nidx_regs:
                nidx_regs[n] = nc.gpsimd.to_reg(n)
            return nidx_regs[n]

        eng_flip = [0]

        def proj_block(step, r, t):
            o = 128 * t
            w = min(128, Nl - o)
            g0 = r * Nl + o
            xt = pools["xt"].tile([D, 128], BF16, tag="projlhs")
            if step == 0:
                nc.sync.dma_start(xt[:, :w], xT_in.ap()[:, g0:g0 + w])
            else:
                k = t // SB
                ko = o - 128 * SB * k
                nc.sync.dma_start(
                    xt[:, :w],
                    xT_ag[k].ap()[D * r:D * (r + 1), ko:ko + w])
            pp = pC.tile([128, RW], F32, tag="pj")
            nc.tensor.matmul(pp[:w, :], xt[:, :w], waug_t[:],
                             start=True, stop=True)
            tb = pools["tbl"].tile([128, RW], F8, tag="tbl")
            if eng_flip[0] % 2 == 0:
                nc.vector.tensor_copy(tb[:w, :], pp[:w, :])
            else:
                nc.scalar.activation(tb[:w, :], pp[:w, :], AF.Copy)
            eng_flip[0] += 1
            nc.sync.dma_start(tables[step].ap()[g0:g0 + w, 0:RW], tb[:w, :])

        def proj_sb(step, k):
            # step-1 projection of all cores' node blocks in superblock k
            for r in range(N_CORES):
                for t in range(k * SB, min((k + 1) * SB, NB)):
                    proj_block(step, r, t)

        for step in range(STEP):
            # ---------------------------------------------- step-0 projection
            # (step-1 projection is emitted interleaved into step 0's
            # superblock loop, gated on the per-superblock AllGathers)
            if step == 0:
                for r in range(N_CORES):
                    for t in range(NB):
                        proj_block(0, r, t)

            # ------------------------------------------------ gather + attn
            x_src = x_in if step == 0 else x_mid
            table = tables[step]
            call_i = 0
            group_i = 0
            for isb in range(NSB):
                blocks = list(range(isb * SB, min((isb + 1) * SB, NB)))
                nb = len(blocks)
                b0 = blocks[0]
                acc = pA.tile([128, SB, 512], F32, tag="acc")
                x4 = pools["blk"].tile([128, SB, D], F32, tag="x4")
                c04 = pools["blk"].tile([128, SB, D], F32, tag="c04")
                nc.sync.dma_start(x4[:, :nb, :], x_src.ap()[:, b0:b0 + nb, :])
                nc.sync.dma_start(c04[:, :nb, :], c0_in.ap()[:, b0:b0 + nb, :])
                # x4p = (1-alpha) * x4 + c0
                x4p = pools["blk"].tile([128, SB, D], F32, tag="x4p")
                nc.vector.scalar_tensor_tensor(
                    x4p[:, :nb, :], x4[:, :nb, :], scal_t[:, 0:1],
                    c04[:, :nb, :], op0=ALU.mult, op1=ALU.add)
                er_sb = {}
                for j, b in enumerate(blocks):
                    xtb = pools["xt"].tile([D, 128], BF16, tag="erlhs")
                    if step == 0:
                        nc.sync.dma_start(
                            xtb[:], xTl_in.ap()[:, 128 * b:128 * (b + 1)])
                    else:
                        nc.sync.dma_start(
                            xtb[:], xT_sh[isb].ap()[:, 128 * j:128 * (j + 1)])
                    nc.tensor.matmul(acc[:, j, 264:264 + H], xtb[:], wr_t[:],
                                     start=True, stop=True)
                    es = pools["sm"].tile([128, H], F8, tag="erblk%d" % j)
                    nc.scalar.activation(es[:], acc[:, j, 264:264 + H], AF.Copy)
                    er_sb[b] = es

                # walk this superblock's calls/groups/chunks
                while call_i < len(meta["calls"]):
                    st, lo, nch = meta["calls"][call_i]
                    if lo >= len(meta["chunk_meta"]) or \
                       meta["chunk_meta"][lo][0] != isb:
                        break
                    n = nch * 128
                    rows = pools["rows"].tile([128, MAX_CALL, TW], F8,
                                              tag="rows")
                    icol = meta["call_cols"][call_i]
                    rows_ap = _ap(rows[:], [TW, nch], [1, TW])
                    tbl_ap = table.ap()[st * st_rows:
                                        min((st + 1) * st_rows, N), :]
                    if not SKIP_GATHER:
                        nc.gpsimd.dma_gather(
                            rows_ap, tbl_ap, idx_t[:, icol:icol + n // 16],
                            num_idxs=n, num_idxs_reg=nidx_reg(n), elem_size=TW,
                            single_packet=SINGLE_PACKET,
                            queue_num=call_i % N_QUEUES)
                    call_i += 1

                    while group_i < len(meta["groups"]):
                        gst, glo_call, g, gs = meta["groups"][group_i]
                        if glo_call != lo:
                            break
                        group_i += 1
                        cc0 = g - lo   # chunk offset within call
                        # transposed one-hot masks (host-precomputed, fp8)
                        mt = pools["mask"].tile([128, GS * 128], F8, tag="mt")
                        nc.sync.dma_start(
                            mt[:, :gs * 128],
                            mt8_in.ap()[:, g * 128:(g + gs) * 128])
                        # dst one-hot m8 (grouped is_equal)
                        m8 = pools["m8p"].tile([128, GS, 128], BF16, tag="m8")
                        nc.vector.tensor_tensor(
                            _ap(m8[:], [128, gs], [1, 128]),
                            _ap(iota_t[:], [0, gs], [1, 128]),
                            _ap(doff_t[:, g:g + gs], [1, gs], [0, 128]),
                            op=ALU.is_equal)
                        # er per edge via fp8 mask matmul
                        er8 = pB.tile([128, GS * H], F32, tag="er8")
                        for k in range(gs):
                            ci = g + k
                            _, _, b = meta["chunk_meta"][ci]
                            nc.tensor.matmul(er8[:, H * k:H * (k + 1)],
                                             mt[:, 128 * k:128 * (k + 1)],
                                             er_sb[b], start=True, stop=True)
                        # t8 = er8 + el (gathered, fp8)
                        t8 = pools["sm"].tile([128, GS * H], BF16, tag="t8")
                        nc.vector.tensor_tensor(
                            t8[:, :gs * H], er8[:, :gs * H],
                            _ap(rows[:, cc0:cc0 + gs, HD:HD + H],
                                [TW, gs], [1, H]),
                            op=ALU.add)
                        lr8 = pools["sm"].tile([128, GS * H], BF16, tag="lr8")
                        nc.vector.scalar_tensor_tensor(
                            lr8[:, :gs * H], t8[:, :gs * H], NEG_SLOPE,
                            t8[:, :gs * H], op0=ALU.mult, op1=ALU.max)
                        rhs8 = pools["rhs"].tile([128, GS, RW], BF16, tag="rhs8")
                        nc.scalar.activation(
                            _ap(rhs8[:], [RW, gs], [1, H]),
                            _ap(lr8[:], [H, gs], [1, H]), AF.Exp)
                        nc.vector.tensor_tensor(
                            _ap(rhs8[:, :, H:RW], [RW, gs], [D, H], [1, D]),
                            _ap(rows[:, cc0:cc0 + gs, 0:HD],
                                [TW, gs], [D, H], [1, D]),
                            _ap(rhs8[:], [RW, gs], [1, H], [0, D]),
                            op=ALU.mult)
                        for k in range(gs):
                            ci = g + k
                            _, _, b = meta["chunk_meta"][ci]
                            j = b - b0
                            nc.tensor.matmul(
                                acc[:, j, 0:RW], m8[:, k, :], rhs8[:, k, :],
                                start=(meta["first"][(isb, b)] == ci),
                                stop=(meta["last"][(isb, b)] == ci),
                                skip_group_check=True)

                # ---- superblock epilogue (batched over blocks)
                smax = pools["sm"].tile([128, SB * H], F32, tag="smax")
                nc.vector.tensor_scalar(
                    _ap(smax[:], [H, nb], [1, H]),
                    _ap(acc[:], [512, nb], [1, H]),
                    1e-30, None, op0=ALU.max)
                srec = pools["sm"].tile([128, SB * H], F32, tag="srec")
                nc.vector.reciprocal(srec[:, :nb * H], smax[:, :nb * H])
                srec2 = pools["sm"].tile([128, SB * H], F32, tag="srec2")
                nc.vector.tensor_scalar(
                    srec2[:, :nb * H], srec[:, :nb * H], scal_t[:, 1:2], None,
                    op0=ALU.mult)
                onorm = pools["big"].tile([128, SB, H, D], F32, tag="onorm")
                nc.vector.tensor_tensor(
                    _ap(onorm[:], [H * D, nb], [D, H], [1, D]),
                    _ap(acc[:, :, H:RW], [512, nb], [D, H], [1, D]),
                    _ap(srec2[:], [H, nb], [1, H], [0, D]),
                    op=ALU.mult)
                red = pools["blk"].tile([128, SB, D], F32, tag="red")
                nc.vector.tensor_reduce(
                    _ap(red[:], [D, nb], [1, D]),
                    _ap(onorm[:], [H * D, nb], [1, D], [D, H]),
                    axis=mybir.AxisListType.X, op=ALU.add)
                xn = pools["blk"].tile([128, SB, D], F32, tag="xn")
                nc.vector.tensor_add(xn[:, :nb, :], x4p[:, :nb, :],
                                     red[:, :nb, :])
                if step < STEP - 1:
                    xnb = pools["blk"].tile([128, SB, D], BF16, tag="xnb")
                    nc.vector.tensor_copy(xnb[:, :nb, :], xn[:, :nb, :])
                    nc.sync.dma_start(x_mid.ap()[:, b0:b0 + nb, :],
                                      xn[:, :nb, :])
                    for j, b in enumerate(blocks):
                        tp = pC.tile([D, 128], BF16, tag="pj")
                        nc.tensor.transpose(tp[:], xnb[:, j, :], ident_t[:])
                        xts = pools["sm"].tile([D, 128], BF16, tag="xts")
                        nc.scalar.activation(xts[:], tp[:], AF.Copy)
                        nc.sync.dma_start(
                            xT_sh[isb].ap()[:, 128 * j:128 * (j + 1)], xts[:])
                    # chunked AllGather of this superblock's updated xT; then
                    # emit step-1 projection for a 2-superblock-earlier AG so
                    # engine FIFO heads never stall on an in-flight collective
                    if not SKIP_COLL:
                        nc.gpsimd.collective_compute(
                            "AllGather", ALU.bypass,
                            replica_groups=[list(range(N_CORES))],
                            ins=[xT_sh[isb].ap()[:]],
                            outs=[xT_ag[isb].ap()[:]])
                    if isb - 2 >= 0:
                        proj_sb(step + 1, isb - 2)
                else:
                    for j, b in enumerate(blocks):
                        w = tails.get(b, 128)
                        nc.sync.dma_start(x_out.ap()[128 * b:128 * b + w, :],
                                          xn[:w, j, :])
            assert call_i == len(meta["calls"]) and \
                group_i == len(meta["groups"])

            if step < STEP - 1:
                for k in range(max(0, NSB - 2), NSB):
                    proj_sb(step + 1, k)

    _split_multi_waits(nc)
    lower_extended_insts(nc)
    return nc


# ----------------------------------------------------------------------------
# entry point
# ----------------------------------------------------------------------------

def kernel(x, x0, src, dst, W, attn_l, attn_r, alpha, lamda, **kw):
    global _last_results
    x = np.asarray(x, np.float32)
    x0 = np.asarray(x0, np.float32)
    src = np.asarray(src)
    dst = np.asarray(dst)
    W = np.asarray(W, np.float32)
    attn_l = np.asarray(attn_l, np.float32)
    attn_r = np.asarray(attn_r, np.float32)
    alpha_f = float(np.asarray(alpha))
    lamda_f = float(np.asarray(lamda))

    N, D = x.shape
    H = attn_l.shape[0]
    assert N % N_CORES == 0
    meta = _plan_and_arrays(src, dst, N)
    Nl, NB = meta["Nl"], meta["NB"]
    NBP = NB * 128

    nc = _build(meta, N, D, H)

    # host-side weight prep
    W3 = W.reshape(D, H, D)
    WL = np.einsum("khd,hd->kh", W3, attn_l)
    WR = np.einsum("khd,hd->kh", W3, attn_r)
    waug = _bf(np.concatenate([W, WL], axis=1))
    wr = _bf(WR)
    iota = _bf(np.tile(np.arange(128, dtype=np.float32)[None, :], (128, 1)))
    ident = _bf(np.eye(128, dtype=np.float32))
    scal = np.zeros((128, 4), np.float32)
    scal[:, 0] = 1.0 - alpha_f
    scal[:, 1] = alpha_f / H
    c0 = (alpha_f * lamda_f) * x0

    d_idx = np.arange(128, dtype=np.float32)
    xT = _bf(x.T).copy()                      # [D, N]
    in_maps = []
    for p in range(N_CORES):
        lo = p * Nl
        xl = np.zeros((NBP, D), np.float32)
        xl[:Nl] = x[lo:lo + Nl]
        c0l = np.zeros((NBP, D), np.float32)
        c0l[:Nl] = c0[lo:lo + Nl]
        # transposed multi-chunk one-hot mask: mt8[d, ci*128+e] =
        # (dst_off(ci, e) == d), fp8 {0,1}
        mt8 = _f8(meta["doff_raw"][p][None, :] == d_idx[:, None])
        in_maps.append({
            "xT_in": np.ascontiguousarray(xT),
            "xTl_in": np.ascontiguousarray(_bf(xl.T)),
            "x_in": np.ascontiguousarray(
                xl.reshape(NB, 128, D).transpose(1, 0, 2)),
            "c0_in": np.ascontiguousarray(
                c0l.reshape(NB, 128, D).transpose(1, 0, 2)),
            "waug_in": waug, "wr_in": wr,
            "iota_in": iota, "ident_in": ident,
            "scal_in": scal,
            "idx_in": np.ascontiguousarray(
                np.tile(meta["idx_wrapped"][p], (8, 1))),
            "doff_in": np.ascontiguousarray(_bf(meta["dstoff"][p])),
            "mt8_in": np.ascontiguousarray(mt8),
        })

    trace = bool(int(os.environ.get("GAT_TRACE", "0")))
    res = run_bass_kernel_spmd(nc, in_maps, core_ids=list(range(N_CORES)),
                               trace=trace,
                               trace_cores=[0] if trace else None,
                               stitch_traces=False)
    _last_results = res
    out = np.concatenate([res.results[p]["x_out"] for p in range(N_CORES)],
                         axis=0)
    return out.astype(np.float32)


# revision 15
# speedup vs baseline: 1.8762x; 1.3243x over previous
"""GAT (graph attention) message-passing kernel for Trainium2, 8 NeuronCores.

Strategy (graph/data parallel, dst-sharded):
  - Nodes are partitioned across 8 cores by destination id (12500 each).
  - Edges are sharded by dst partition, sorted by (dst-block, src-subtable),
    and padded so every core runs an identical (SPMD) program.
  - Per step, every core projects ALL nodes (h = x @ [W | W@attn_l]) into an
    fp8 row table in its HBM ([h(256B) | el(4B) | pad] @ 512B stride), then
    indirect-gathers h[src] rows per edge (dma_gather, 4 SWDGE queues round
    robin so descriptor generation runs on all 4 Q7 core pairs), computes
    attention scores (er via host-precomputed transposed one-hot masks
    streamed from HBM as fp8 + gathered el added on DVE), and accumulates
    [softmax-denominator | weighted message sum] into per-dst-block PSUM
    with mask matmuls on TensorE.  The dst one-hot masks are built on DVE
    with per-chunk tensor_scalar(is_equal) (4x mode).
  - Block epilogue: normalize by the segment sum, head-mean, residual update.
  - Between the 2 conv steps, the updated x (transposed, bf16) is AllGathered
    across the 8 cores.
"""

import os
import math
import numpy as np
import ml_dtypes

import concourse.bass as bass
import concourse.tile as tile
import concourse.mybir as mybir
from concourse import library_config
from concourse.library_overlay import lower_extended_insts
from concourse.bass_utils import run_bass_kernel_spmd

BF16 = mybir.dt.bfloat16
F32 = mybir.dt.float32
F8 = mybir.dt.float8e4
I16 = mybir.dt.int16
AF = mybir.ActivationFunctionType
ALU = mybir.AluOpType

NEG_SLOPE = 0.2
STEP = int(os.environ.get("GAT_STEPS", "2"))
SKIP_COLL = bool(int(os.environ.get("GAT_SKIP_COLL", "0")))
SKIP_GATHER = bool(int(os.environ.get("GAT_SKIP_GATHER", "0")))
N_QUEUES = int(os.environ.get("GAT_QUEUES", "4"))
SINGLE_PACKET = bool(int(os.environ.get("GAT_SINGLE_PACKET", "0")))
N_CORES = 8
SB = 2            # blocks per superblock (PSUM accumulators alive at once)
MAX_CALL = int(os.environ.get("GAT_MAX_CALL", "8"))  # chunks per dma_gather call
GS = 8            # chunks per elementwise batch group
ST_MAX_ROWS = 25000   # subtable rows (int16 gather index limit)
PAD_IDX = int(os.environ.get("GAT_PAD_IDX", "0"))

_last_results = None  # BassKernelResults stash for test harness


def _bf(x):
    return np.asarray(x, np.float32).astype(ml_dtypes.bfloat16)


def _f8(x):
    return np.asarray(x, np.float32).astype(ml_dtypes.float8_e4m3fn)


# ----------------------------------------------------------------------------
# host-side preprocessing
# ----------------------------------------------------------------------------

def _plan_and_arrays(src, dst, N):
    """Shard/sort/pad edges; build the shared chunk plan and per-core arrays."""
    Nl = N // N_CORES
    NB = (Nl + 127) // 128
    NSB = (NB + SB - 1) // SB
    NST = max(1, math.ceil(N / ST_MAX_ROWS))
    st_rows = math.ceil(N / NST)

    core = dst // Nl
    percore = []
    for p in range(N_CORES):
        sel = np.nonzero(core == p)[0]
        s = src[sel].astype(np.int64)
        d = (dst[sel] - p * Nl).astype(np.int64)
        blk = d >> 7
        st = s // st_rows
        order = np.lexsort((s, st, blk))
        percore.append((s[order], d[order], blk[order], st[order]))

    counts = np.zeros((N_CORES, NB, NST), np.int64)
    for p in range(N_CORES):
        _, _, blk, st = percore[p]
        np.add.at(counts, (p, blk, st), 1)
    nchunks = (counts.max(axis=0) + 127) // 128          # [NB, NST]

    # canonical chunk emission order.  One call per (b, st) run so that all
    # idx padding is TRAILING within its call: the gather ucode trims
    # trailing negative idxs before descriptor generation, skipping the
    # padding's Pool + DMA cost entirely.
    chunk_meta = []   # (isb, st, b) per chunk
    calls = []        # (st, chunk_lo, n_chunks)
    for isb in range(NSB):
        blocks = range(isb * SB, min((isb + 1) * SB, NB))
        for st in range(NST):
            for b in blocks:
                run_lo = len(chunk_meta)
                for _ in range(int(nchunks[b, st])):
                    chunk_meta.append((isb, st, b))
                n = len(chunk_meta) - run_lo
                o = run_lo
                while n > 0:
                    take = min(n, MAX_CALL)
                    calls.append((st, o, take))
                    o += take
                    n -= take
    NCH = len(chunk_meta)

    # first/last chunk index per (isb, b) for PSUM start/stop flags
    first = {}
    last = {}
    for ci, (isb, st, b) in enumerate(chunk_meta):
        key = (isb, b)
        if key not in first:
            first[key] = ci
        last[key] = ci

    # per-core edge arrays in padded chunk order.  NOTE: idx padding must be
    # >= 0 (a valid row; masks zero its contribution).  Negative padding
    # desyncs the gather's decode-side ring reservation (sized from the
    # num_idxs register) from the Q7 impl's trimmed descriptor count,
    # leaving stale descriptors that hang the DMA engines.
    idx_all = np.full((N_CORES, NCH * 128), PAD_IDX, np.int16)
    doff_all = np.full((N_CORES, NCH * 128), 255.0, np.float32)
    for p in range(N_CORES):
        s, d, blk, st = percore[p]
        # build run boundaries of the (blk, st)-sorted edge list
        runs = {}
        i = 0
        M = len(s)
        while i < M:
            k = (blk[i], st[i])
            j = i
            while j < M and blk[j] == k[0] and st[j] == k[1]:
                j += 1
            runs[k] = (i, j)
            i = j
        cursor = {k: v[0] for k, v in runs.items()}
        for ci, (isb, t, b) in enumerate(chunk_meta):
            base = ci * 128
            k = (b, t)
            if k in runs:
                lo = cursor[k]
                hi = min(lo + 128, runs[k][1])
                n = hi - lo
                cursor[k] = hi
                if n > 0:
                    idx_all[p, base:base + n] = (s[lo:hi] - t * st_rows).astype(np.int16)
                    doff_all[p, base:base + n] = (d[lo:hi] - b * 128).astype(np.float32)
        for k, (lo, hi) in runs.items():
            assert cursor[k] == hi, "edge run not fully consumed"

    # gather-call wrapped idx layout: per call [16, n/16], concat on free axis
    idxw_cols = NCH * 8
    idx_wrapped = np.zeros((N_CORES, 16, idxw_cols), np.int16)
    col = 0
    call_cols = []
    for (t, lo, nch) in calls:
        n = nch * 128
        for p in range(N_CORES):
            seg = idx_all[p, lo * 128: lo * 128 + n]
            idx_wrapped[p, :, col:col + n // 16] = seg.reshape(-1, 16).T
        call_cols.append(col)
        col += n // 16
    assert col == idxw_cols

    # dstoff [128, NCH]: partition = edge-in-chunk
    doff = doff_all.reshape(N_CORES, NCH, 128).transpose(0, 2, 1)

    groups = []
    for (t, lo, nch) in calls:
        g = lo
        while g < lo + nch:
            take = min(GS, lo + nch - g)
            groups.append((t, lo, g, take))  # (st, call_lo, group_lo, size)
            g += take

    return dict(Nl=Nl, NB=NB, NSB=NSB, NST=NST, st_rows=st_rows, NCH=NCH,
                chunk_meta=chunk_meta, calls=calls, call_cols=call_cols,
                groups=groups, first=first, last=last,
                idx_wrapped=idx_wrapped, dstoff=doff, doff_raw=doff_all,
                idxw_cols=idxw_cols)


# ----------------------------------------------------------------------------
# device program
# ----------------------------------------------------------------------------

def _split_multi_waits(nc):
    """walrus codegen only accepts one sync-wait per instruction; hoist any
    extra waits onto same-engine NOPs inserted right before the instruction."""
    n_id = 0
    for f in nc.m.functions:
        for blk in f.blocks:
            out = []
            for ins in blk.instructions:
                si = ins.sync_info
                if si is not None and len(si.on_wait) > 1 \
                        and ins.engine is not None:
                    waits = list(si.on_wait)
                    for w in waits[:-1]:
                        nop = mybir.InstNoOp(name=f"I-wsplit-{n_id}", ins=[],
                                             outs=[])
                        n_id += 1
                        nop.engine = ins.engine
                        nop.sync_info = mybir.SyncInfo(on_wait=[w],
                                                       on_update=[])
                        nc.inst_map[nop.name] = nop
                        out.append(nop)
                    ins.sync_info = mybir.SyncInfo(on_wait=[waits[-1]],
                                                   on_update=list(si.on_update))
                out.append(ins)
            blk.instructions = out

def _ap(base, *dims):
    """Rebuild AP with the same tensor/offset/partition dim, custom free dims."""
    return bass.AP(base.tensor, base.offset,
                   [list(base.ap[0])] + [list(d) for d in dims])


def _build(meta, N, D, H):
    Nl, NB, NSB, NST = meta["Nl"], meta["NB"], meta["NSB"], meta["NST"]
    st_rows = meta["st_rows"]
    NBP = NB * 128
    HD = H * D            # 256
    RW = HD + H           # 260 elems (h | el), fp8 -> 260B used
    TW = 512              # fp8 table row stride: 512B (gather elem size)

    nc = bass.Bass("TRN2", target_bir_lowering=False, debug=False,
                   enable_asserts=False, num_devices=N_CORES,
                   num_swdge_queues=N_QUEUES,
                   dynamic_dma_scratch_size=32768)

    # ---- DRAM tensors
    xT_in = nc.dram_tensor("xT_in", [D, N], BF16, kind="ExternalInput")
    xTl_in = nc.dram_tensor("xTl_in", [D, NBP], BF16, kind="ExternalInput")
    x_in = nc.dram_tensor("x_in", [128, NB, D], F32, kind="ExternalInput")
    c0_in = nc.dram_tensor("c0_in", [128, NB, D], F32, kind="ExternalInput")
    waug_in = nc.dram_tensor("waug_in", [D, RW], BF16, kind="ExternalInput")
    wr_in = nc.dram_tensor("wr_in", [D, H], BF16, kind="ExternalInput")
    iota_in = nc.dram_tensor("iota_in", [128, 128], BF16, kind="ExternalInput")
    ident_in = nc.dram_tensor("ident_in", [128, 128], BF16, kind="ExternalInput")
    scal_in = nc.dram_tensor("scal_in", [128, 4], F32, kind="ExternalInput")
    idx_in = nc.dram_tensor("idx_in", [128, meta["idxw_cols"]], I16,
                            kind="ExternalInput")
    doff_in = nc.dram_tensor("doff_in", [128, meta["NCH"]], BF16,
                             kind="ExternalInput")
    mt8_in = nc.dram_tensor("mt8_in", [128, meta["NCH"] * 128], F8,
                            kind="ExternalInput")

    # double-buffered row table: step-1 projection writes overlap step-0
    # gather reads, so each step gathers from its own table
    tables = [nc.dram_tensor("table%d" % s, [N, TW], F8, kind="Internal")
              for s in range(STEP)]
    x_mid = nc.dram_tensor("x_mid", [128, NB, D], F32, kind="Internal")
    # per-superblock xT shards + AllGather outputs (chunked collective so
    # step-1 projection can start as soon as each superblock's AG lands)
    sb_cols = [128 * (min((k + 1) * SB, NB) - k * SB) for k in range(NSB)]
    xT_sh = [nc.dram_tensor("xT_sh%d" % k, [D, sb_cols[k]], BF16,
                            kind="Internal") for k in range(NSB)]
    xT_ag = [nc.dram_tensor("xT_ag%d" % k, [D * N_CORES, sb_cols[k]], BF16,
                            kind="Internal", addr_space="Shared")
             for k in range(NSB)]
    x_out = nc.dram_tensor("x_out", [Nl, D], F32, kind="ExternalOutput")

    from contextlib import ExitStack
    with tile.TileContext(nc) as tc, ExitStack() as es_:
        nc.gpsimd.load_library(library_config.mlp)
        cp = es_.enter_context(tc.tile_pool(name="consts", bufs=1))
        pools = {}
        for nm, bufs in [("xt", 8), ("rows", 8), ("mask", 6), ("m8p", 4),
                         ("rhs", 6), ("sm", 8), ("tbl", 8), ("blk", 6),
                         ("big", 4)]:
            pools[nm] = es_.enter_context(tc.tile_pool(name=nm, bufs=bufs))
        pA = es_.enter_context(tc.tile_pool(name="pacc", bufs=2, space="PSUM"))
        pB = es_.enter_context(tc.tile_pool(name="per8", bufs=1, space="PSUM"))
        pC = es_.enter_context(tc.tile_pool(name="ppj", bufs=3, space="PSUM"))

        # ---- load constants
        iota_t = cp.tile([128, 128], BF16, tag="iota")
        ident_t = cp.tile([128, 128], BF16, tag="ident")
        waug_t = cp.tile([D, RW], BF16, tag="waug")
        wr_t = cp.tile([D, H], BF16, tag="wr")
        scal_t = cp.tile([128, 4], F32, tag="scal")
        idx_t = cp.tile([128, meta["idxw_cols"]], I16, tag="idx")
        doff_t = cp.tile([128, meta["NCH"]], BF16, tag="doff")
        for t, s in [(iota_t, iota_in), (ident_t, ident_in),
                     (waug_t, waug_in), (wr_t, wr_in), (scal_t, scal_in),
                     (idx_t, idx_in), (doff_t, doff_in)]:
            nc.sync.dma_start(t[:], s.ap()[:])

        tails = {NB - 1: Nl - 128 * (NB - 1)}
        nidx_regs = {}

        def nidx_reg(n):
            if n not in nidx_regs:
                nidx_regs[n] = nc.gpsimd.to_reg(n)
            return nidx_regs[n]

        eng_flip = [0]

        def proj_block(step, r, t):
            o = 128 * t
            w = min(128, Nl - o)
            g0 = r * Nl + o
            xt = pools["xt"].tile([D, 128], BF16, tag="projlhs")
            if step == 0:
                nc.sync.dma_start(xt[:, :w], xT_in.ap()[:, g0:g0 + w])
            else:
                k = t // SB
                ko = o - 128 * SB * k
                nc.sync.dma_start(
                    xt[:, :w],
                    xT_ag[k].ap()[D * r:D * (r + 1), ko:ko + w])
            pp = pC.tile([128, RW], F32, tag="pj")
            nc.tensor.matmul(pp[:w, :], xt[:, :w], waug_t[:],
                             start=True, stop=True)
            tb = pools["tbl"].tile([128, RW], F8, tag="tbl")
            if eng_flip[0] % 2 == 0:
                nc.vector.tensor_copy(tb[:w, :], pp[:w, :])
            else:
                nc.scalar.activation(tb[:w, :], pp[:w, :], AF.Copy)
            eng_flip[0] += 1
            nc.sync.dma_start(tables[step].ap()[g0:g0 + w, 0:RW], tb[:w, :])

        def proj_sb(step, k):
            # step-1 projection of all cores' node blocks in superblock k
            for r in range(N_CORES):
                for t in range(k * SB, min((k + 1) * SB, NB)):
                    proj_block(step, r, t)

        for step in range(STEP):
            # ---------------------------------------------- step-0 projection
            # (step-1 projection is emitted interleaved into step 0's
            # superblock loop, gated on the per-superblock AllGathers)
            if step == 0:
                for r in range(N_CORES):
                    for t in range(NB):
                        proj_block(0, r, t)

            # ------------------------------------------------ gather + attn
            x_src = x_in if step == 0 else x_mid
            table = tables[step]
            call_i = 0
            group_i = 0
            for isb in range(NSB):
                blocks = list(range(isb * SB, min((isb + 1) * SB, NB)))
                nb = len(blocks)
                b0 = blocks[0]
                acc = pA.tile([128, SB, 512], F32, tag="acc")
                er8w = pB.tile([128, 512], F32, tag="er8w")
                x4 = pools["blk"].tile([128, SB, D], F32, tag="x4")
                c04 = pools["blk"].tile([128, SB, D], F32, tag="c04")
                nc.sync.dma_start(x4[:, :nb, :], x_src.ap()[:, b0:b0 + nb, :])
                nc.sync.dma_start(c04[:, :nb, :], c0_in.ap()[:, b0:b0 + nb, :])
                # x4p = (1-alpha) * x4 + c0
                x4p = pools["blk"].tile([128, SB, D], F32, tag="x4p")
                nc.vector.scalar_tensor_tensor(
                    x4p[:, :nb, :], x4[:, :nb, :], scal_t[:, 0:1],
                    c04[:, :nb, :], op0=ALU.mult, op1=ALU.add)
                er_sb = {}
                for j, b in enumerate(blocks):
                    xtb = pools["xt"].tile([D, 128], BF16, tag="erlhs")
                    if step == 0:
                        nc.sync.dma_start(
                            xtb[:], xTl_in.ap()[:, 128 * b:128 * (b + 1)])
                    else:
                        nc.sync.dma_start(
                            xtb[:], xT_sh[isb].ap()[:, 128 * j:128 * (j + 1)])
                    nc.tensor.matmul(acc[:, j, 264:264 + H], xtb[:], wr_t[:],
                                     start=True, stop=True)
                    es = pools["sm"].tile([128, H], F8, tag="erblk%d" % j)
                    nc.scalar.activation(es[:], acc[:, j, 264:264 + H], AF.Copy)
                    er_sb[b] = es

                # walk this superblock's calls/groups/chunks
                while call_i < len(meta["calls"]):
                    st, lo, nch = meta["calls"][call_i]
                    if lo >= len(meta["chunk_meta"]) or \
                       meta["chunk_meta"][lo][0] != isb:
                        break
                    n = nch * 128
                    rows = pools["rows"].tile([128, MAX_CALL, TW], F8,
                                              tag="rows")
                    icol = meta["call_cols"][call_i]
                    rows_ap = _ap(rows[:], [TW, nch], [1, TW])
                    tbl_ap = table.ap()[st * st_rows:
                                        min((st + 1) * st_rows, N), :]
                    if not SKIP_GATHER:
                        nc.gpsimd.dma_gather(
                            rows_ap, tbl_ap, idx_t[:, icol:icol + n // 16],
                            num_idxs=n, num_idxs_reg=nidx_reg(n), elem_size=TW,
                            single_packet=SINGLE_PACKET,
                            queue_num=call_i % N_QUEUES)
                    call_i += 1

                    while group_i < len(meta["groups"]):
                        gst, glo_call, g, gs = meta["groups"][group_i]
                        if glo_call != lo:
                            break
                        group_i += 1
                        cc0 = g - lo   # chunk offset within call
                        # transposed one-hot masks (host-precomputed, fp8)
                        mt = pools["mask"].tile([128, GS * 128], F8, tag="mt")
                        nc.sync.dma_start(
                            mt[:, :gs * 128],
                            mt8_in.ap()[:, g * 128:(g + gs) * 128])
                        # dst one-hot m8 (grouped is_equal)
                        m8 = pools["m8p"].tile([128, GS, 128], BF16, tag="m8")
                        nc.vector.tensor_tensor(
                            _ap(m8[:], [128, gs], [1, 128]),
                            _ap(iota_t[:], [0, gs], [1, 128]),
                            _ap(doff_t[:, g:g + gs], [1, gs], [0, 128]),
                            op=ALU.is_equal)
                        # er per edge via fp8 mask matmul, into a
                        # rotating window of the dedicated er8 PSUM bank
                        ew = 32 * (group_i % 16)
                        er8 = er8w[:, ew:ew + GS * H]
                        for k in range(gs):
                            ci = g + k
                            _, _, b = meta["chunk_meta"][ci]
                            nc.tensor.matmul(er8[:, H * k:H * (k + 1)],
                                             mt[:, 128 * k:128 * (k + 1)],
                                             er_sb[b], start=True, stop=True,
                                             skip_group_check=True)
                        # t8 = er8 + el (gathered, fp8)
                        t8 = pools["sm"].tile([128, GS * H], BF16, tag="t8")
                        nc.vector.tensor_tensor(
                            t8[:, :gs * H], er8[:, :gs * H],
                            _ap(rows[:, cc0:cc0 + gs, HD:HD + H],
                                [TW, gs], [1, H]),
                            op=ALU.add)
                        lr8 = pools["sm"].tile([128, GS * H], BF16, tag="lr8")
                        nc.vector.scalar_tensor_tensor(
                            lr8[:, :gs * H], t8[:, :gs * H], NEG_SLOPE,
                            t8[:, :gs * H], op0=ALU.mult, op1=ALU.max)
                        rhs8 = pools["rhs"].tile([128, GS, RW], BF16, tag="rhs8")
                        nc.scalar.activation(
                            _ap(rhs8[:], [RW, gs], [1, H]),
                            _ap(lr8[:], [H, gs], [1, H]), AF.Exp)
                        nc.vector.tensor_tensor(
                            _ap(rhs8[:, :, H:RW], [RW, gs], [D, H], [1, D]),
                            _ap(rows[:, cc0:cc0 + gs, 0:HD],
                                [TW, gs], [D, H], [1, D]),
                            _ap(rhs8[:], [RW, gs], [1, H], [0, D]),
                            op=ALU.mult)
                        for k in range(gs):
                            ci = g + k
                            _, _, b = meta["chunk_meta"][ci]
                            j = b - b0
                            nc.tensor.matmul(
                                acc[:, j, 0:RW], m8[:, k, :], rhs8[:, k, :],
                                start=(meta["first"][(isb, b)] == ci),
                                stop=(meta["last"][(isb, b)] == ci),
                                skip_group_check=True)

                # ---- superblock epilogue (batched over blocks)
                smax = pools["sm"].tile([128, SB * H], F32, tag="smax")
                nc.vector.tensor_scalar(
                    _ap(smax[:], [H, nb], [1, H]),
                    _ap(acc[:], [512, nb], [1, H]),
                    1e-30, None, op0=ALU.max)
                srec = pools["sm"].tile([128, SB * H], F32, tag="srec")
                nc.vector.reciprocal(srec[:, :nb * H], smax[:, :nb * H])
                srec2 = pools["sm"].tile([128, SB * H], F32, tag="srec2")
                nc.vector.tensor_scalar(
                    srec2[:, :nb * H], srec[:, :nb * H], scal_t[:, 1:2], None,
                    op0=ALU.mult)
                onb = pools["big"].tile([128, SB, H, D], BF16, tag="onb")
                nc.scalar.activation(
                    _ap(onb[:], [H * D, nb], [1, H * D]),
                    _ap(acc[:, :, H:RW], [512, nb], [1, H * D]), AF.Copy)
                onorm = pools["big"].tile([128, SB, H, D], BF16, tag="onorm")
                nc.vector.tensor_tensor(
                    _ap(onorm[:], [H * D, nb], [D, H], [1, D]),
                    _ap(onb[:], [H * D, nb], [D, H], [1, D]),
                    _ap(srec2[:], [H, nb], [1, H], [0, D]),
                    op=ALU.mult)
                red = pools["blk"].tile([128, SB, D], F32, tag="red")
                nc.vector.tensor_reduce(
                    _ap(red[:], [D, nb], [1, D]),
                    _ap(onorm[:], [H * D, nb], [1, D], [D, H]),
                    axis=mybir.AxisListType.X, op=ALU.add)
                xn = pools["blk"].tile([128, SB, D], F32, tag="xn")
                nc.vector.tensor_add(xn[:, :nb, :], x4p[:, :nb, :],
                                     red[:, :nb, :])
                if step < STEP - 1:
                    xnb = pools["blk"].tile([128, SB, D], BF16, tag="xnb")
                    nc.vector.tensor_copy(xnb[:, :nb, :], xn[:, :nb, :])
                    nc.sync.dma_start(x_mid.ap()[:, b0:b0 + nb, :],
                                      xn[:, :nb, :])
                    for j, b in enumerate(blocks):
                        tp = pC.tile([D, 128], BF16, tag="pj")
                        nc.tensor.transpose(tp[:], xnb[:, j, :], ident_t[:])
                        xts = pools["sm"].tile([D, 128], BF16, tag="xts")
                        nc.scalar.activation(xts[:], tp[:], AF.Copy)
                        nc.sync.dma_start(
                            xT_sh[isb].ap()[:, 128 * j:128 * (j + 1)], xts[:])
                    # chunked AllGather of this superblock's updated xT; then
                    # emit step-1 projection for a 2-superblock-earlier AG so
                    # engine FIFO heads never stall on an in-flight collective
                    if not SKIP_COLL:
                        nc.gpsimd.collective_compute(
                            "AllGather", ALU.bypass,
                            replica_groups=[list(range(N_CORES))],
                            ins=[xT_sh[isb].ap()[:]],
                            outs=[xT_ag[isb].ap()[:]])
                    if isb - 2 >= 0:
                        proj_sb(step + 1, isb - 2)
                else:
                    for j, b in enumerate(blocks):
                        w = tails.get(b, 128)
                        nc.sync.dma_start(x_out.ap()[128 * b:128 * b + w, :],
                                          xn[:w, j, :])
            assert call_i == len(meta["calls"]) and \
                group_i == len(meta["groups"])

            if step < STEP - 1:
                for k in range(max(0, NSB - 2), NSB):
                    proj_sb(step + 1, k)

    _split_multi_waits(nc)
    lower_extended_insts(nc)
    return nc


# ----------------------------------------------------------------------------
# entry point
# ----------------------------------------------------------------------------

def kernel(x, x0, src, dst, W, attn_l, attn_r, alpha, lamda, **kw):
    global _last_results
    x = np.asarray(x, np.float32)
    x0 = np.asarray(x0, np.float32)
    src = np.asarray(src)
    dst = np.asarray(dst)
    W = np.asarray(W, np.float32)
    attn_l = np.asarray(attn_l, np.float32)
    attn_r = np.asarray(attn_r, np.float32)
    alpha_f = float(np.asarray(alpha))
    lamda_f = float(np.asarray(lamda))

    N, D = x.shape
    H = attn_l.shape[0]
    assert N % N_CORES == 0
    meta = _plan_and_arrays(src, dst, N)
    Nl, NB = meta["Nl"], meta["NB"]
    NBP = NB * 128

    nc = _build(meta, N, D, H)

    # host-side weight prep
    W3 = W.reshape(D, H, D)
    WL = np.einsum("khd,hd->kh", W3, attn_l)
    WR = np.einsum("khd,hd->kh", W3, attn_r)
    waug = _bf(np.concatenate([W, WL], axis=1))
    wr = _bf(WR)
    iota = _bf(np.tile(np.arange(128, dtype=np.float32)[None, :], (128, 1)))
    ident = _bf(np.eye(128, dtype=np.float32))
    scal = np.zeros((128, 4), np.float32)
    scal[:, 0] = 1.0 - alpha_f
    scal[:, 1] = alpha_f / H
    c0 = (alpha_f * lamda_f) * x0

    d_idx = np.arange(128, dtype=np.float32)
    xT = _bf(x.T).copy()                      # [D, N]
    in_maps = []
    for p in range(N_CORES):
        lo = p * Nl
        xl = np.zeros((NBP, D), np.float32)
        xl[:Nl] = x[lo:lo + Nl]
        c0l = np.zeros((NBP, D), np.float32)
        c0l[:Nl] = c0[lo:lo + Nl]
        # transposed multi-chunk one-hot mask: mt8[d, ci*128+e] =
        # (dst_off(ci, e) == d), fp8 {0,1}
        mt8 = _f8(meta["doff_raw"][p][None, :] == d_idx[:, None])
        in_maps.append({
            "xT_in": np.ascontiguousarray(xT),
            "xTl_in": np.ascontiguousarray(_bf(xl.T)),
            "x_in": np.ascontiguousarray(
                xl.reshape(NB, 128, D).transpose(1, 0, 2)),
            "c0_in": np.ascontiguousarray(
                c0l.reshape(NB, 128, D).transpose(1, 0, 2)),
            "waug_in": waug, "wr_in": wr,
            "iota_in": iota, "ident_in": ident,
            "scal_in": scal,
            "idx_in": np.ascontiguousarray(
                np.tile(meta["idx_wrapped"][p], (8, 1))),
            "doff_in": np.ascontiguousarray(_bf(meta["dstoff"][p])),
            "mt8_in": np.ascontiguousarray(mt8),
        })

    trace = bool(int(os.environ.get("GAT_TRACE", "0")))
    res = run_bass_kernel_spmd(nc, in_maps, core_ids=list(range(N_CORES)),
                               trace=trace,
                               trace_cores=[0] if trace else None,
                               stitch_traces=False)
    _last_results = res
    out = np.concatenate([res.results[p]["x_out"] for p in range(N_CORES)],
                         axis=0)
    return out.astype(np.float32)
